# revision 33
# baseline (speedup 1.0000x reference)
"""STSPBlock Trainium2 kernel.

Structure (per core, batch-sharded B=16 -> 8 cores x B=2):
  partitions p = b*64 + channel for all activation tensors.

  - conv0+bn+LIF-input-scale folded into one K=73 im2col matmul.
    x is split HOST-side into hi (11 mantissa bits, exactly
    representable under the PE's float32r moving-operand rounding) and
    lo (residual): 36 hi tap rows + 36 lo tap rows + 1 ones row
    (bias), so the fp32r matmul is numerically exact for the conv.
    Taps are DMA'd from DRAM copies of x_hi/x_lo laid out as
    zero-PADDED 66x66 planes, so no edge-correction matmuls are
    needed. The LIF state add (1-c0)*v rides the same PSUM group via a
    scaled-identity fp32r matmul.

  - Spikes are computed on the SCALAR engine as s' = Sign(u - 1) in
    {-1,+1} ("sign encoding", s = (s'+1)/2); every consumer is linear
    in s, so the affine fix is folded host-side into the next conv's
    weights/bias, the feat-transform consts, and the y weights. The
    out0 pool pad cells are preset to -4 (the sign encoding of 0).
    Sign/Exp/Ln/Copy all live in one ACT table set
    (natural_log_exp_and_others), preloaded once.

  - reset v' = (s' < 0) * u on DVE; avgpool via paired adds (values
    become integer sums in [-4,4] -> exact under fp32r).

  - BETA=0 => S-state is just alpha each step. alpha is applied by
    scaling the block-diag node conv weights; the sign-encoding
    constant term (alpha-dependent, spatially uniform thanks to the -4
    padding) enters PSUM via a tiny K=2 matmul whose rhs is alpha
    broadcast with a 0-stride AP.

  - y = affine combination of sign spikes on DVE.

All bn/LIF/sigmoid parameter folding is done host-side from the actual
input values at call time, so the kernel is fully general.
"""

import numpy as np

import concourse.bass as bass
import concourse.bacc as bacc
import concourse.mybir as mybir
from concourse.tile import TileContext
from concourse.bass_utils import run_bass_kernel_spmd

FP = mybir.dt.float32
FPR = mybir.dt.float32r
Alu = mybir.AluOpType
Act = mybir.ActivationFunctionType

T, BFULL, CIN, H, W = 8, 16, 2, 64, 64
CO, NN, HEADS = 64, 4, 4
HP, WP = 32, 32
BC = 2                    # batch per core
NCORES = 8
EPS = 1e-5
DECAY = 0.6
HD = CO // HEADS          # 16
PL = 66 * 66              # padded plane size
NPL = 4                   # planes per timestep (b, ci)

ACT_SET_NLE = None  # index of the Sign+Exp+Ln+Copy ACT table set


# ----------------------------------------------------------------- host consts
def _host_consts(conv0_w, bn0_g, bn0_b, bn0_m, bn0_v, lif0_w,
                 convs_w, bns_g, bns_b, bns_m, bns_v, lifs_w,
                 ft_w, ft_b, gat_w, gat_a, out_weights):
    f32 = np.float32
    sig = lambda z: 1.0 / (1.0 + np.exp(-z.astype(np.float64)))
    c0 = f32(sig(lif0_w))
    cn = sig(lifs_w).astype(f32)          # [3]

    s0c = (bn0_g / np.sqrt(bn0_v + EPS)).astype(f32)
    bias0 = ((bn0_b - bn0_m * s0c) * c0).astype(f32)
    W0f = (conv0_w * s0c[:, None, None, None] * c0).astype(f32)  # [64,2,3,3]

    # fp32r rounds BOTH matmul operands to ~12 mantissa bits. Split
    # weights and x into hi (11-bit, exactly representable) + lo parts
    # and keep the three cross products Whi*xhi + Whi*xlo + Wlo*xhi
    # (dropped Wlo*xlo term is ~2^-22 relative): the conv is exact.
    def tr11(a):
        return (np.ascontiguousarray(a, f32).view(np.uint32)
                & np.uint32(0xFFFFF000)).view(f32)

    W0hi = tr11(W0f)
    W0lo = (W0f - W0hi).astype(f32)
    b0hi = tr11(bias0)
    b0lo = (bias0 - b0hi).astype(f32)

    # w0bd [110,128]: rows 0/1 = bias hi/lo (ones taps); rows 2-37 =
    # Whi (vs x_hi rows), 38-73 = Whi (vs x_lo), 74-109 = Wlo (vs x_hi
    # again). col m = b*64+co.
    w0bd = np.zeros((110, 128), f32)
    w0bd[0, 0:64] = b0hi
    w0bd[0, 64:128] = b0hi
    w0bd[1, 0:64] = b0lo
    w0bd[1, 64:128] = b0lo
    for dy in range(3):
        for dx in range(3):
            for b in range(2):
                for ci in range(2):
                    p = dy * 12 + dx * 4 + b * 2 + ci
                    w0bd[2 + p, b * 64:(b + 1) * 64] = W0hi[:, ci, dy, dx]
                    w0bd[38 + p, b * 64:(b + 1) * 64] = W0hi[:, ci, dy, dx]
                    w0bd[74 + p, b * 64:(b + 1) * 64] = W0lo[:, ci, dy, dx]

    i0 = ((1.0 - c0) * np.eye(128)).astype(f32)

    sncol = (bns_g / np.sqrt(bns_v + EPS)).astype(f32)            # [3,64]
    biasn_raw = (bns_b - bns_m * sncol).astype(f32)               # [3,64]
    # 0.25 = avgpool fold; extra 0.5 = sign-encoding decode s=(s'+1)/2
    Wf = (convs_w * sncol[:, :, None, None, None] * 0.25).astype(f32)
    Wh = (Wf * 0.5).astype(f32)

    # wnod [3, 9, 128, 128]: per (node, tap) block-diag lhsT over b
    wnod = np.zeros((3, 9, 128, 128), f32)
    for n in range(3):
        for dy in range(3):
            for dx in range(3):
                k = dy * 3 + dx
                blk = Wh[n, :, :, dy, dx].T    # [ci, co]
                wnod[n, k, 0:64, 0:64] = blk
                wnod[n, k, 64:128, 64:128] = blk

    in3 = np.stack([(1.0 - cn[n]) * np.eye(128) for n in range(3)]).astype(f32)
    biasn = np.concatenate([np.tile(cn[n] * biasn_raw[n], 2)
                            for n in range(3)]).reshape(1, 384).astype(f32)

    # apad [2, 3*128]: sign-encoding uniform term 2*sum_taps(Wf)[n,co];
    # rhs = alpha*cn broadcast, so the term becomes alpha*cn*2*tapsum.
    tapsum = Wf.sum(axis=(2, 3, 4))            # [3, 64]
    apad = np.zeros((2, 3 * 128), f32)
    for n in range(3):
        for b in range(2):
            apad[b, n * 128 + b * 64:n * 128 + (b + 1) * 64] = 2.0 * tapsum[n]

    def bd(m):  # block-diag [128,128] of m.T twice ([co,ci] -> lhsT)
        z = np.zeros((128, 128), f32)
        z[0:64, 0:64] = m.T
        z[64:128, 64:128] = m.T
        return z

    # feat transform: f04 = 0.4*relu(ftw @ mean + ftb), sign-decode and
    # the 0.4 trace factor folded:  mean0 = (0.125/1024)*S0sum + 0.5,
    # meann = (0.5/1024)*Snsum + 0.5.
    ftmm = np.stack([bd(ft_w * (0.4 * 0.125 / 1024.0)),
                     bd(ft_w * (0.4 * 0.5 / 1024.0))])
    ftb_f = (0.4 * (ft_b + 0.5 * ft_w.sum(axis=1))).astype(f32)
    ftb2 = np.tile(ftb_f, 2).reshape(128, 1).astype(f32)
    gwbd = bd(gat_w).astype(f32)

    # ga1/ga2 [128, 8]: in p=(b, c') c'=h*16+d ; out m = b*4+h
    ga1 = np.zeros((128, 8), f32)
    ga2 = np.zeros((128, 8), f32)
    for b in range(2):
        for h in range(HEADS):
            for d in range(HD):
                ga1[b * 64 + h * 16 + d, b * 4 + h] = gat_a[h, d]
                ga2[b * 64 + h * 16 + d, b * 4 + h] = gat_a[h, HD + d]

    # ghbd [8,2]: p=(b,h) -> col b ; carries 0.5(sym)*0.25(mean h)/0.01(temp)
    ghbd = np.zeros((8, 2), f32)
    for b in range(2):
        ghbd[b * 4:(b + 1) * 4, b] = 12.5

    gbc = np.zeros((2, 128), f32)
    gbc[0, 0:64] = 1.0
    gbc[1, 64:128] = 1.0

    # cnrow4 [2,4]: col 0 unused (node 0 has no conv), cols 1-3 = cn
    cnrow4 = np.zeros((2, 4), f32)
    cnrow4[:, 1:4] = cn[None, :]

    def cols(stk):  # [k,128,128] -> [128, k*128]
        return np.ascontiguousarray(
            np.transpose(stk, (1, 0, 2)).reshape(128, -1))

    return dict(w0bd=w0bd, i0=i0, wnod=cols(wnod.reshape(27, 128, 128)),
                in3=cols(in3), biasn=biasn, apad=apad,
                ftmm=cols(ftmm), ftb2=ftb2, gwbd=gwbd,
                ga1=ga1, ga2=ga2, ghbd=ghbd, gbc=gbc, cnrow4=cnrow4)


CONST_SHAPES = dict(w0bd=(110, 128), i0=(128, 128), wnod=(128, 27 * 128),
                    in3=(128, 3 * 128), biasn=(1, 384), apad=(2, 3 * 128),
                    ftmm=(128, 2 * 128), ftb2=(128, 1), gwbd=(128, 128),
                    ga1=(128, 8), ga2=(128, 8), ghbd=(8, 2), gbc=(2, 128),
                    cnrow4=(2, 4))
# consts that feed the big float32r matmuls
FPR_CONSTS = {"w0bd", "i0", "wnod", "in3", "biasn", "apad", "gbc"}


def _act_set_id():
    global ACT_SET_NLE
    if ACT_SET_NLE is None:
        from concourse.hw_specs import get_activation_tables
        for i, name in enumerate(get_activation_tables("gen3")):
            if name == "natural_log_exp_and_others":
                ACT_SET_NLE = i
                break
        assert ACT_SET_NLE is not None
    return ACT_SET_NLE


# ------------------------------------------------------------------ the module
def build_nc(nt=T, yw=(0.125, 0.5, 0.5, 0.5), yc=1.0):
    nc = bacc.Bacc(None, target_bir_lowering=False)
    xh = nc.declare_dram_parameter("xh", [T, BC, CIN, H, W], FPR,
                                   isOutput=False)
    xl = nc.declare_dram_parameter("xl", [T, BC, CIN, H, W], FPR,
                                   isOutput=False)
    cst = {k: nc.declare_dram_parameter(
               k, list(v), FPR if k in FPR_CONSTS else FP, isOutput=False)
           for k, v in CONST_SHAPES.items()}
    y = nc.declare_dram_parameter("y", [T, BC, CO, HP, WP], FP, isOutput=True)
    xlinH = nc.dram_tensor("xlinH", [T * NPL * PL], FPR)
    xlinL = nc.dram_tensor("xlinL", [T * NPL * PL], FPR)

    with TileContext(nc) as tc:
        with (
            tc.tile_pool(name="consts", bufs=1) as cpool,
            tc.tile_pool(name="state", bufs=1) as spool,
            tc.tile_pool(name="im", bufs=1) as impool,
            tc.tile_pool(name="work", bufs=2) as wpool,
            tc.tile_pool(name="sw", bufs=1) as swpool,
            tc.tile_pool(name="tiny", bufs=3) as tpool,
            tc.tile_pool(name="pconv", bufs=2, space="PSUM") as ps_conv,
            tc.tile_pool(name="pnode", bufs=2, space="PSUM") as ps_node,
            tc.tile_pool(name="ptiny", bufs=3, space="PSUM") as ps_tiny,
        ):
            # ---- preload the one ACT table set we use (Sign/Exp/Ln/Copy)
            ld = mybir.InstLoadActFuncSet(
                name=f"I-{nc.next_id()}", ins=[], outs=[],
                act_func_set_id=_act_set_id())
            nc.scalar.add_instruction(ld)

            # ---- consts to SBUF
            csb = {}
            for k, shp in CONST_SHAPES.items():
                t_ = cpool.tile(list(shp), FPR if k in FPR_CONSTS else FP,
                                tag=k)
                nc.sync.dma_start(t_[:], cst[k][:])
                csb[k] = t_

            zcol = cpool.tile([128, 1], FP, tag="zcol")
            nc.vector.memset(zcol[:], 0.0)
            ocol = cpool.tile([128, 1], FP, tag="ocol")
            nc.vector.memset(ocol[:], 1.0)
            m4col = cpool.tile([128, 1], FP, tag="m4col")
            nc.vector.memset(m4col[:], -4.0)
            mcol = cpool.tile([128, 1], FP, tag="mcol")   # ACT bias -1
            nc.vector.memset(mcol[:], -1.0)
            actb = cpool.tile([128, 2], FP, tag="actb")
            nc.vector.memset(actb[:, 0:1], 0.0)
            nc.vector.memset(actb[:, 1:2], 1e-6)

            def bcfill(dst, src2d, *shape):
                nc.vector.tensor_copy(
                    dst, bass.AP(tensor=src2d.tensor, offset=src2d.offset,
                                 ap=[list(src2d.ap[0])]
                                 + [[0, s] for s in shape]))

            # ---- states
            v0a = spool.tile([128, 4096], FPR, tag="v0a")
            v0b = spool.tile([128, 4096], FPR, tag="v0b")
            vna = spool.tile([128, 3072], FPR, tag="vna")
            vnb = spool.tile([128, 3072], FPR, tag="vnb")
            Tt = spool.tile([128, 4], FP, tag="Tt")
            bcfill(v0a[:], zcol[:, 0:1], 4096)
            bcfill(vna[:], zcol[:, 0:1], 3072)
            nc.vector.memset(Tt[:], 0.0)

            # ---- persistent padded out0 tiles, pads preset to -4 once
            o0tiles = []
            for nm in ("o0A", "o0B"):
                o0t = spool.tile([128, 34 * 34], FPR, tag=nm)
                o0v = o0t[:].rearrange("p (h w) -> p h w", h=34)
                bcfill(o0v[:, 0, :], m4col[:, 0:1], 34)
                bcfill(o0v[:, 33, :], m4col[:, 0:1], 34)
                bcfill(o0v[:, 1:33, 0:1], m4col[:, 0:1], 32, 1)
                bcfill(o0v[:, 1:33, 33:34], m4col[:, 0:1], 32, 1)
                o0tiles.append(o0t)

            # ---- zero plane row (single partition, one full 66x66 plane)
            zplane = cpool.tile([1, PL], FPR, tag="zplane")
            bcfill(zplane[:], zcol[0:1, 0:1], PL)

            # ---- build padded x planes in DRAM (one-time):
            # zero-fill every plane, then overwrite the 64x64 interior.
            # Tile's DRAM dep tracking orders the overlapping writes.
            for xsrc, xdst in ((xh, xlinH), (xl, xlinL)):
                zp = zplane[:]
                nc.sync.dma_start(
                    bass.AP(tensor=xdst, offset=0,
                            ap=[[PL, T * NPL], [1, PL]]),
                    bass.AP(tensor=zp.tensor, offset=zp.offset,
                            ap=[[1, 1], [0, T * NPL], [1, PL]]))
                nc.sync.dma_start(
                    bass.AP(tensor=xdst, offset=67,
                            ap=[[PL, T * NPL], [66, 64], [1, 64]]),
                    bass.AP(tensor=xsrc, offset=0,
                            ap=[[4096, T * NPL], [64, 64], [1, 64]]))

            # ---- im2col tiles (row 0 = ones, set once; the rest streamed)
            imA = impool.tile([110, 4096], FPR, tag="imA")
            imB = impool.tile([110, 4096], FPR, tag="imB")
            for imt in (imA, imB):
                bcfill(imt[0:2, :], ocol[0:2, 0:1], 4096)

            def colmat(name, j):
                return csb[name][:, j * 128:(j + 1) * 128]
            ftb2ap = csb["ftb2"][:]

            for t in range(nt):
                v0o, v0n = (v0a, v0b) if t % 2 == 0 else (v0b, v0a)
                vno, vnn = (vna, vnb) if t % 2 == 0 else (vnb, vna)
                im = imA if t % 2 == 0 else imB
                out0p = o0tiles[t % 2]
                o0r = out0p[:].rearrange("p (h w) -> p h w", h=34)

                # ---- im2col DMA: 36 hi rows + 36 lo rows
                for dy in range(3):
                    for dx in range(3):
                        for base, xlin_ in ((2, xlinH), (38, xlinL),
                                            (74, xlinH)):
                            p0 = base + dy * 12 + dx * 4
                            nc.sync.dma_start(
                                im[p0:p0 + 4, :],
                                bass.AP(tensor=xlin_,
                                        offset=(t * NPL * PL + dy * 66
                                                + dx),
                                        ap=[[PL, NPL],
                                            [66, 64], [1, 64]]))

                # ---- conv0 + LIF0, 8 chunks of 512 (8 h-rows each)
                p1 = wpool.tile([128, 2048], FP, tag="p1")
                for c in range(8):
                    sl = slice(c * 512, (c + 1) * 512)
                    ps = ps_conv.tile([128, 512], FP, tag="pc")
                    nc.tensor.matmul(ps[:], csb["w0bd"][:], im[:, sl],
                                     start=True, stop=False)
                    nc.tensor.matmul(ps[:], csb["i0"][:], v0o[:, sl],
                                     start=False, stop=True,
                                     skip_group_check=True)
                    # s' = Sign(u-1) on ACT ; v' = (s'<0)*u on DVE
                    s0c = wpool.tile([128, 512], FP, tag="s0c")
                    nc.scalar.activation(s0c[:], ps[:], Act.Sign,
                                         bias=mcol[:, 0:1], scale=1.0)
                    nc.vector.scalar_tensor_tensor(
                        v0n[:, sl], s0c[:], 0.0, ps[:], Alu.is_lt, Alu.mult)
                    s0r = s0c[:].rearrange("p (h w) -> p h w", h=8)
                    p1r = p1[:].rearrange("p (h w) -> p h w", h=64)
                    nc.vector.tensor_tensor(
                        p1r[:, c * 8:(c + 1) * 8, :],
                        s0r[:, :, 0::2], s0r[:, :, 1::2], Alu.add)

                # ---- pool rows into padded out0 (S in [-4,4]) + f0 sum
                f0sum = tpool.tile([128, 1], FP, tag="f0sum")
                p1v = p1[:].rearrange("p (h w) -> p h w", h=64)
                nc.vector.tensor_tensor(
                    o0r[:, 1:33, 1:33], p1v[:, 0::2, :], p1v[:, 1::2, :],
                    Alu.add)
                nc.vector.tensor_reduce(f0sum[:], o0r[:, 1:33, 1:33],
                                        mybir.AxisListType.XY, Alu.add)

                # ---- f04 = 0.4*relu(ft @ mean + ftb)   (folded consts)
                psf0 = ps_tiny.tile([128, 1], FP, tag="gt")
                nc.tensor.matmul(psf0[:], colmat("ftmm", 0), f0sum[:],
                                 start=True, stop=True)
                f04 = tpool.tile([128, 1], FP, tag="f04")
                nc.vector.tensor_scalar(f04[:], psf0[:], ftb2ap, 0.0,
                                        Alu.add, op1=Alu.max)

                # ---- trace row0 pre-update
                nc.vector.scalar_tensor_tensor(
                    Tt[:, 0:1], Tt[:, 0:1], DECAY, f04[:], Alu.mult, Alu.add)

                # ================= graph math =================
                def tiny(tag, p_, f_, dt_=FP):
                    return tpool.tile([p_, f_], dt_, tag=tag, name=tag)

                psg = ps_tiny.tile([128, 4], FP, tag="gt")
                nc.tensor.matmul(psg[:], csb["gwbd"][:], Tt[:],
                                 start=True, stop=True)
                hpc = tiny("hpc", 128, 4)
                nc.vector.tensor_copy(hpc[:], psg[:])

                pse1 = ps_tiny.tile([8, 4], FP, tag="gt")
                nc.tensor.matmul(pse1[:], csb["ga1"][:], hpc[:],
                                 start=True, stop=True)
                e1t = tiny("e1t", 8, 4)
                nc.vector.tensor_copy(e1t[:], pse1[:])
                pse2 = ps_tiny.tile([8, 4], FP, tag="gt")
                nc.tensor.matmul(pse2[:], csb["ga2"][:], hpc[:],
                                 start=True, stop=True)
                e2t = tiny("e2t", 8, 4)
                nc.vector.tensor_copy(e2t[:], pse2[:])

                def reap(ap_, tail):
                    dims = [list(d) for d in ap_.ap][:-1] + tail
                    return bass.AP(tensor=ap_.tensor, offset=ap_.offset,
                                   ap=dims)

                def bc_n(ap_):  # [p,4] -> free (n,m): n varies, m bcast
                    return reap(ap_, [[1, 4], [0, 4]])

                def bc_m(ap_):  # free (n,m): n bcast, m varies
                    return reap(ap_, [[0, 4], [1, 4]])

                es = tiny("es", 8, 16)
                nc.vector.tensor_tensor(es[:], bc_n(e1t[:]), bc_m(e2t[:]),
                                        Alu.add)
                es2 = tiny("es2", 8, 16)
                nc.vector.tensor_scalar_mul(es2[:], es[:], 0.2)
                el = tiny("el", 8, 16)
                nc.vector.tensor_tensor(el[:], es[:], es2[:], Alu.max)

                psE = ps_tiny.tile([2, 16], FP, tag="gt")
                nc.tensor.matmul(psE[:], csb["ghbd"][:], el[:],
                                 start=True, stop=True)
                Ec = tiny("Ec", 2, 16)
                nc.vector.tensor_copy(Ec[:], psE[:])

                def tr_nm(ap_):  # read transposed over (n,m)
                    return reap(ap_, [[1, 4], [4, 4]])

                L = tiny("L", 2, 16)
                nc.vector.tensor_tensor(L[:], Ec[:], tr_nm(Ec[:]), Alu.add)
                Lr = L[:].rearrange("p (n m) -> p n m", n=4)
                mx = tiny("mx", 2, 4)
                nc.vector.tensor_reduce(mx[:], Lr, mybir.AxisListType.X,
                                        Alu.max)
                xm = tiny("xm", 2, 16)
                nc.vector.tensor_tensor(xm[:], L[:], bc_n(mx[:]), Alu.subtract)
                ex = tiny("ex", 2, 16)
                nc.scalar.activation(ex[:], xm[:], Act.Exp,
                                     bias=actb[0:2, 0:1])
                sm = tiny("sm", 2, 4)
                exr = ex[:].rearrange("p (n m) -> p n m", n=4)
                nc.vector.tensor_reduce(sm[:], exr, mybir.AxisListType.X,
                                        Alu.add)
                rc = tiny("rc", 2, 4)
                nc.vector.reciprocal(rc[:], sm[:])
                S = tiny("S", 2, 16)
                nc.vector.tensor_tensor(S[:], ex[:], bc_n(rc[:]), Alu.mult)

                Sr = S[:].rearrange("p (n m) -> p n m", n=4)
                lo = tiny("lo", 2, 8)
                lor = lo[:].rearrange("p (n m) -> p n m", n=4)
                hi = tiny("hi", 2, 8)
                hir = hi[:].rearrange("p (n m) -> p n m", n=4)
                nc.vector.tensor_tensor(lor, Sr[:, :, 0::2], Sr[:, :, 1::2],
                                        Alu.min)
                nc.vector.tensor_tensor(hir, Sr[:, :, 0::2], Sr[:, :, 1::2],
                                        Alu.max)
                kth = tiny("kth", 2, 4)
                l2 = tiny("l2", 2, 4)
                nc.vector.tensor_tensor(l2[:], lor[:, :, 0], lor[:, :, 1],
                                        Alu.max)
                h2 = tiny("h2", 2, 4)
                nc.vector.tensor_tensor(h2[:], hir[:, :, 0], hir[:, :, 1],
                                        Alu.min)
                nc.vector.tensor_tensor(kth[:], l2[:], h2[:], Alu.min)
                msk = tiny("msk", 2, 16)
                nc.vector.tensor_tensor(msk[:], S[:], bc_n(kth[:]), Alu.is_ge)
                Sp = tiny("Sp", 2, 16)
                nc.vector.tensor_tensor(Sp[:], S[:], msk[:], Alu.mult)

                A2 = tiny("A2", 2, 16)
                nc.vector.tensor_tensor(A2[:], Sp[:], tr_nm(Sp[:]), Alu.add)
                rs = tiny("rs", 2, 4)
                A2r = A2[:].rearrange("p (n m) -> p n m", n=4)
                nc.vector.tensor_reduce(rs[:], A2r, mybir.AxisListType.X,
                                        Alu.add)
                lnd = tiny("lnd", 2, 4)
                nc.scalar.activation(lnd[:], rs[:], Act.Ln,
                                     bias=actb[0:2, 1:2], scale=0.5)
                q = tiny("q", 2, 4)
                nc.scalar.activation(q[:], lnd[:], Act.Exp, scale=-0.5,
                                     bias=actb[0:2, 0:1])

                t1 = tiny("t1", 2, 16)
                nc.vector.tensor_tensor(t1[:], A2[:], bc_n(q[:]), Alu.mult)
                OPt = tiny("OPt", 2, 16)
                nc.vector.scalar_tensor_tensor(OPt[:], t1[:], 0.5, bc_m(q[:]),
                                               Alu.mult, Alu.mult)
                col0 = reap(OPt[:], [[0, 4], [4, 4]])
                t2 = tiny("t2", 2, 16)
                nc.vector.tensor_tensor(t2[:], OPt[:], col0, Alu.mult)
                af = tiny("af", 2, 4)
                t2r = t2[:].rearrange("p (n m) -> p n m", n=4)
                nc.vector.tensor_reduce(af[:], t2r, mybir.AxisListType.X,
                                        Alu.add)
                # al3f [2,4] fpr: cols 1-3 = alpha*cn, col 0 garbage*0=0
                al3f = tiny("al3f", 2, 4, FPR)
                nc.vector.tensor_tensor(al3f[:], af[:], csb["cnrow4"][:],
                                        Alu.mult)
                psb = ps_tiny.tile([128, 4], FP, tag="gt")
                nc.tensor.matmul(psb[:], csb["gbc"][:], al3f[:],
                                 start=True, stop=True)
                aap = tiny("aap", 128, 4)
                nc.vector.tensor_copy(aap[:], psb[:])

                # ================= node path =================
                sn = wpool.tile([128, 3072], FP, tag="sn")
                snsum = tpool.tile([128, 3], FP, tag="snsum")
                snsumB = tpool.tile([128, 3], FP, tag="snsumB")
                sw = [swpool.tile([128, 9 * 128], FPR, tag=f"sw{n}",
                                  name=f"sw{n}") for n in range(3)]
                for n in range(3):
                    nc.vector.tensor_scalar_mul(
                        sw[n][:],
                        csb["wnod"][:, n * 9 * 128:(n + 1) * 9 * 128],
                        aap[:, n + 1:n + 2])
                for n in range(3):
                    for c in range(2):
                        psn = ps_node.tile([128, 512], FP, tag="pn")
                        for k in range(9):
                            dy, dx = k // 3, k % 3
                            rhs = o0r[:, dy + 16 * c: dy + 16 * c + 16,
                                      dx:dx + 32]
                            nc.tensor.matmul(psn[:],
                                             sw[n][:, k * 128:(k + 1) * 128],
                                             rhs, start=(k == 0), stop=False)
                        nc.tensor.matmul(
                            psn[:], csb["biasn"][0:1, n * 128:(n + 1) * 128],
                            im[0:1, c * 512:(c + 1) * 512],
                            start=False, stop=False)
                        nc.tensor.matmul(
                            psn[:], csb["apad"][:, n * 128:(n + 1) * 128],
                            reap(al3f[:, n + 1:n + 2], [[0, 512]]),
                            start=False, stop=False, skip_group_check=True)
                        nc.tensor.matmul(
                            psn[:], colmat("in3", n),
                            vno[:, n * 1024 + c * 512:
                                n * 1024 + (c + 1) * 512],
                            start=False, stop=True)
                        sl = slice(n * 1024 + c * 512,
                                   n * 1024 + (c + 1) * 512)
                        nc.scalar.activation(
                            sn[:, sl], psn[:], Act.Sign, bias=mcol[:, 0:1],
                            accum_out=(snsum if c == 0
                                       else snsumB)[:, n:n + 1])
                        nc.vector.scalar_tensor_tensor(
                            vnn[:, sl], sn[:, sl], 0.0, psn[:],
                            Alu.is_lt, Alu.mult)

                # ---- feats + trace update
                psf = ps_tiny.tile([128, 3], FP, tag="gt")
                nc.tensor.matmul(psf[:], colmat("ftmm", 1), snsum[:],
                                 start=True, stop=False)
                nc.tensor.matmul(psf[:], colmat("ftmm", 1), snsumB[:],
                                 start=False, stop=True)
                fn04 = tpool.tile([128, 3], FP, tag="fn04")
                nc.vector.tensor_scalar(fn04[:], psf[:], ftb2ap, 0.0,
                                        Alu.add, op1=Alu.max)
                nc.vector.scalar_tensor_tensor(
                    Tt[:, 0:1], Tt[:, 0:1], DECAY, f04[:], Alu.mult, Alu.add)
                nc.vector.scalar_tensor_tensor(
                    Tt[:, 1:4], Tt[:, 1:4], DECAY, fn04[:], Alu.mult, Alu.add)

                # ================= output y (DVE) =================
                # y = yw0*S0 + sum_n ywn*sn' + yc
                ysb = wpool.tile([128, 1024], FP, tag="ysb")
                nc.vector.tensor_scalar(ysb[:], o0r[:, 1:33, 1:33],
                                        yw[0], yc, Alu.mult, op1=Alu.add)
                for n in range(3):
                    nc.vector.scalar_tensor_tensor(
                        ysb[:], sn[:, n * 1024:(n + 1) * 1024], yw[n + 1],
                        ysb[:], Alu.mult, Alu.add)
                nc.sync.dma_start(
                    bass.AP(tensor=y, offset=t * BC * CO * 1024,
                            ap=[[1024, 128], [1, 1024]]),
                    ysb[:])
    if not nc.is_finalized():
        nc.finalize()
    return nc


_NC_CACHE = {}


def _get_nc(nt=T, yw=(0.125, 0.5, 0.5, 0.5), yc=1.0):
    key = (nt, tuple(float(v) for v in yw), float(yc))
    if key not in _NC_CACHE:
        _NC_CACHE[key] = build_nc(nt, yw, yc)
    return _NC_CACHE[key]


def _split_hi_lo(x):
    """Split fp32 array into hi (11 mantissa bits, fp32r-exact) + lo."""
    xb = np.ascontiguousarray(x, np.float32)
    hi = (xb.view(np.uint32) & np.uint32(0xFFFFF000)).view(np.float32)
    lo = (xb - hi).astype(np.float32)
    return hi, lo


def kernel(**inputs):
    x = np.asarray(inputs["x"], np.float32)
    consts = _host_consts(
        inputs["conv0_w"], inputs["bn0_g"], inputs["bn0_b"], inputs["bn0_m"],
        inputs["bn0_v"], inputs["lif0_w"], inputs["convs_w"], inputs["bns_g"],
        inputs["bns_b"], inputs["bns_m"], inputs["bns_v"], inputs["lifs_w"],
        inputs["ft_w"], inputs["ft_b"], inputs["gat_w"], inputs["gat_a"],
        inputs["out_weights"])
    consts = {k: np.ascontiguousarray(v, np.float32)
              for k, v in consts.items()}
    sigw = 1.0 / (1.0 + np.exp(-np.asarray(inputs["out_weights"], np.float64)))
    yw = (float(sigw[0]) / 8.0, float(sigw[1]) / 2.0, float(sigw[2]) / 2.0,
          float(sigw[3]) / 2.0)
    yc = float(sigw[0] / 2.0 + (sigw[1] + sigw[2] + sigw[3]) / 2.0)
    nc = _get_nc(T, yw, yc)
    xhi, xlo = _split_hi_lo(x)
    core_ids = list(range(NCORES))
    in_maps = []
    for k in core_ids:
        m = dict(consts)
        m["xh"] = np.ascontiguousarray(xhi[:, k * BC:(k + 1) * BC])
        m["xl"] = np.ascontiguousarray(xlo[:, k * BC:(k + 1) * BC])
        in_maps.append(m)
    res = run_bass_kernel_spmd(nc, in_maps, core_ids).results
    out = np.concatenate([res[k]["y"] for k in core_ids], axis=1)
    return out.astype(np.float32)


# revision 36
# speedup vs baseline: 1.4828x; 1.4828x over previous
"""STSPBlock Trainium2 kernel.

Structure (per core, batch-sharded B=16 -> 8 cores x B=2):
  partitions p = b*64 + channel for all activation tensors.

  - conv0+bn+LIF-input-scale folded into one K=73 im2col matmul.
    x is split HOST-side into hi (11 mantissa bits, exactly
    representable under the PE's float32r moving-operand rounding) and
    lo (residual): 36 hi tap rows + 36 lo tap rows + 1 ones row
    (bias), so the fp32r matmul is numerically exact for the conv.
    Taps are DMA'd from DRAM copies of x_hi/x_lo laid out as
    zero-PADDED 66x66 planes, so no edge-correction matmuls are
    needed. The LIF state add (1-c0)*v rides the same PSUM group via a
    scaled-identity fp32r matmul.

  - Spikes are computed on the SCALAR engine as s' = Sign(u - 1) in
    {-1,+1} ("sign encoding", s = (s'+1)/2); every consumer is linear
    in s, so the affine fix is folded host-side into the next conv's
    weights/bias, the feat-transform consts, and the y weights. The
    out0 pool pad cells are preset to -4 (the sign encoding of 0).
    Sign/Exp/Ln/Copy all live in one ACT table set
    (natural_log_exp_and_others), preloaded once.

  - reset v' = (s' < 0) * u on DVE; avgpool via paired adds (values
    become integer sums in [-4,4] -> exact under fp32r).

  - BETA=0 => S-state is just alpha each step. alpha is applied by
    scaling the block-diag node conv weights; the sign-encoding
    constant term (alpha-dependent, spatially uniform thanks to the -4
    padding) enters PSUM via a tiny K=2 matmul whose rhs is alpha
    broadcast with a 0-stride AP.

  - y = affine combination of sign spikes on DVE.

All bn/LIF/sigmoid parameter folding is done host-side from the actual
input values at call time, so the kernel is fully general.
"""

import numpy as np

import concourse.bass as bass
import concourse.bacc as bacc
import concourse.mybir as mybir
from concourse.tile import TileContext
from concourse.bass_utils import run_bass_kernel_spmd

FP = mybir.dt.float32
FPR = mybir.dt.float32r
Alu = mybir.AluOpType
Act = mybir.ActivationFunctionType

T, BFULL, CIN, H, W = 8, 16, 2, 64, 64
CO, NN, HEADS = 64, 4, 4
HP, WP = 32, 32
BC = 2                    # batch per core
NCORES = 8
EPS = 1e-5
DECAY = 0.6
HD = CO // HEADS          # 16
PL = 66 * 66              # padded plane size
NPL = 4                   # planes per timestep (b, ci)

ACT_SET_NLE = None  # index of the Sign+Exp+Ln+Copy ACT table set


# ----------------------------------------------------------------- host consts
def _host_consts(conv0_w, bn0_g, bn0_b, bn0_m, bn0_v, lif0_w,
                 convs_w, bns_g, bns_b, bns_m, bns_v, lifs_w,
                 ft_w, ft_b, gat_w, gat_a, out_weights):
    f32 = np.float32
    sig = lambda z: 1.0 / (1.0 + np.exp(-z.astype(np.float64)))
    c0 = f32(sig(lif0_w))
    cn = sig(lifs_w).astype(f32)          # [3]

    s0c = (bn0_g / np.sqrt(bn0_v + EPS)).astype(f32)
    bias0 = ((bn0_b - bn0_m * s0c) * c0).astype(f32)
    W0f = (conv0_w * s0c[:, None, None, None] * c0).astype(f32)  # [64,2,3,3]

    # fp32r rounds BOTH matmul operands to ~12 mantissa bits. Split
    # weights and x into hi (11-bit, exactly representable) + lo parts
    # and keep the three cross products Whi*xhi + Whi*xlo + Wlo*xhi
    # (dropped Wlo*xlo term is ~2^-22 relative): the conv is exact.
    def tr11(a):
        return (np.ascontiguousarray(a, f32).view(np.uint32)
                & np.uint32(0xFFFFF000)).view(f32)

    W0hi = tr11(W0f)
    W0lo = (W0f - W0hi).astype(f32)
    b0hi = tr11(bias0)
    b0lo = (bias0 - b0hi).astype(f32)

    # w0bd [110, 9*128]: rows 0/1 = bias hi/lo (ones taps); rows 2-37 =
    # Whi (vs x_hi rows), 38-73 = Whi (vs x_lo), 74-109 = Wlo (vs x_hi
    # again). col m = v*128 + b*64 + co. Variant 0 = full conv+bias;
    # variants 1-8 are edge-garbage fixes (run as plain fp32 matmuls,
    # which exactly cancel the fp32r-exact garbage products):
    # 1: dx=0 taps, 2: dx=2, 3: dy=0, 4: dy=2, 5-8: corner add-backs.
    w0bd = np.zeros((110, 9 * 128), f32)
    w0bd[0, 0:64] = b0hi
    w0bd[0, 64:128] = b0hi
    w0bd[1, 0:64] = b0lo
    w0bd[1, 64:128] = b0lo

    def put(v, dy, dx, sgn):
        for b in range(2):
            for ci in range(2):
                p = dy * 12 + dx * 4 + b * 2 + ci
                c0_, c1_ = v * 128 + b * 64, v * 128 + (b + 1) * 64
                w0bd[2 + p, c0_:c1_] = sgn * W0hi[:, ci, dy, dx]
                w0bd[38 + p, c0_:c1_] = sgn * W0hi[:, ci, dy, dx]
                w0bd[74 + p, c0_:c1_] = sgn * W0lo[:, ci, dy, dx]

    for dy in range(3):
        for dx in range(3):
            put(0, dy, dx, 1.0)
    for dy in range(3):
        put(1, dy, 0, -1.0)
        put(2, dy, 2, -1.0)
    for dx in range(3):
        put(3, 0, dx, -1.0)
        put(4, 2, dx, -1.0)
    put(5, 0, 0, 1.0)
    put(6, 0, 2, 1.0)
    put(7, 2, 0, 1.0)
    put(8, 2, 2, 1.0)

    i0 = ((1.0 - c0) * np.eye(128)).astype(f32)

    sncol = (bns_g / np.sqrt(bns_v + EPS)).astype(f32)            # [3,64]
    biasn_raw = (bns_b - bns_m * sncol).astype(f32)               # [3,64]
    # 0.25 = avgpool fold; extra 0.5 = sign-encoding decode s=(s'+1)/2
    Wf = (convs_w * sncol[:, :, None, None, None] * 0.25).astype(f32)
    Wh = (Wf * 0.5).astype(f32)

    # wnod [3, 9, 128, 128]: per (node, tap) block-diag lhsT over b
    wnod = np.zeros((3, 9, 128, 128), f32)
    for n in range(3):
        for dy in range(3):
            for dx in range(3):
                k = dy * 3 + dx
                blk = Wh[n, :, :, dy, dx].T    # [ci, co]
                wnod[n, k, 0:64, 0:64] = blk
                wnod[n, k, 64:128, 64:128] = blk

    in3 = np.stack([(1.0 - cn[n]) * np.eye(128) for n in range(3)]).astype(f32)
    biasn = np.concatenate([np.tile(cn[n] * biasn_raw[n], 2)
                            for n in range(3)]).reshape(1, 384).astype(f32)

    # apad [2, 3*128]: sign-encoding uniform term 2*sum_taps(Wf)[n,co];
    # rhs = alpha*cn broadcast, so the term becomes alpha*cn*2*tapsum.
    tapsum = Wf.sum(axis=(2, 3, 4))            # [3, 64]
    apad = np.zeros((2, 3 * 128), f32)
    for n in range(3):
        for b in range(2):
            apad[b, n * 128 + b * 64:n * 128 + (b + 1) * 64] = 2.0 * tapsum[n]

    def bd(m):  # block-diag [128,128] of m.T twice ([co,ci] -> lhsT)
        z = np.zeros((128, 128), f32)
        z[0:64, 0:64] = m.T
        z[64:128, 64:128] = m.T
        return z

    # feat transform: f04 = 0.4*relu(ftw @ mean + ftb), sign-decode and
    # the 0.4 trace factor folded:  mean0 = (0.125/1024)*S0sum + 0.5,
    # meann = (0.5/1024)*Snsum + 0.5.
    ftmm = np.stack([bd(ft_w * (0.4 * 0.125 / 1024.0)),
                     bd(ft_w * (0.4 * 0.5 / 1024.0))])
    ftb_f = (0.4 * (ft_b + 0.5 * ft_w.sum(axis=1))).astype(f32)
    ftb2 = np.tile(ftb_f, 2).reshape(128, 1).astype(f32)
    gwbd = bd(gat_w).astype(f32)

    # ga1/ga2 [128, 8]: in p=(b, c') c'=h*16+d ; out m = b*4+h
    ga1 = np.zeros((128, 8), f32)
    ga2 = np.zeros((128, 8), f32)
    for b in range(2):
        for h in range(HEADS):
            for d in range(HD):
                ga1[b * 64 + h * 16 + d, b * 4 + h] = gat_a[h, d]
                ga2[b * 64 + h * 16 + d, b * 4 + h] = gat_a[h, HD + d]

    # ghbd [8,2]: p=(b,h) -> col b ; carries 0.5(sym)*0.25(mean h)/0.01(temp)
    ghbd = np.zeros((8, 2), f32)
    for b in range(2):
        ghbd[b * 4:(b + 1) * 4, b] = 12.5

    gbc = np.zeros((2, 128), f32)
    gbc[0, 0:64] = 1.0
    gbc[1, 64:128] = 1.0

    # cnrow4 [2,4]: col 0 unused (node 0 has no conv), cols 1-3 = cn
    cnrow4 = np.zeros((2, 4), f32)
    cnrow4[:, 1:4] = cn[None, :]

    def cols(stk):  # [k,128,128] -> [128, k*128]
        return np.ascontiguousarray(
            np.transpose(stk, (1, 0, 2)).reshape(128, -1))

    return dict(w0bd=w0bd, i0=i0, wnod=cols(wnod.reshape(27, 128, 128)),
                in3=cols(in3), biasn=biasn, apad=apad,
                ftmm=cols(ftmm), ftb2=ftb2, gwbd=gwbd,
                ga1=ga1, ga2=ga2, ghbd=ghbd, gbc=gbc, cnrow4=cnrow4)


CONST_SHAPES = dict(w0bd=(110, 9 * 128), i0=(128, 128), wnod=(128, 27 * 128),
                    in3=(128, 3 * 128), biasn=(1, 384), apad=(2, 3 * 128),
                    ftmm=(128, 2 * 128), ftb2=(128, 1), gwbd=(128, 128),
                    ga1=(128, 8), ga2=(128, 8), ghbd=(8, 2), gbc=(2, 128),
                    cnrow4=(2, 4))
# consts that feed the big float32r matmuls
FPR_CONSTS = {"w0bd", "i0", "wnod", "in3", "biasn", "apad", "gbc"}


def _act_set_id():
    global ACT_SET_NLE
    if ACT_SET_NLE is None:
        from concourse.hw_specs import get_activation_tables
        for i, name in enumerate(get_activation_tables("gen3")):
            if name == "natural_log_exp_and_others":
                ACT_SET_NLE = i
                break
        assert ACT_SET_NLE is not None
    return ACT_SET_NLE


# ------------------------------------------------------------------ the module
def build_nc(nt=T, yw=(0.125, 0.5, 0.5, 0.5), yc=1.0):
    nc = bacc.Bacc(None, target_bir_lowering=False)
    xh = nc.declare_dram_parameter("xh", [T, BC, CIN, H, W], FPR,
                                   isOutput=False)
    xl = nc.declare_dram_parameter("xl", [T, BC, CIN, H, W], FPR,
                                   isOutput=False)
    cst = {k: nc.declare_dram_parameter(
               k, list(v), FPR if k in FPR_CONSTS else FP, isOutput=False)
           for k, v in CONST_SHAPES.items()}
    y = nc.declare_dram_parameter("y", [T, BC, CO, HP, WP], FP, isOutput=True)
    xlinH = nc.dram_tensor("xlinH", [T * 16384 + 256], FPR)
    xlinL = nc.dram_tensor("xlinL", [T * 16384 + 256], FPR)

    with TileContext(nc) as tc:
        with (
            tc.tile_pool(name="consts", bufs=1) as cpool,
            tc.tile_pool(name="state", bufs=1) as spool,
            tc.tile_pool(name="im", bufs=1) as impool,
            tc.tile_pool(name="work", bufs=2) as wpool,
            tc.tile_pool(name="sw", bufs=1) as swpool,
            tc.tile_pool(name="tiny", bufs=3) as tpool,
            tc.tile_pool(name="pconv", bufs=2, space="PSUM") as ps_conv,
            tc.tile_pool(name="pnode", bufs=2, space="PSUM") as ps_node,
            tc.tile_pool(name="ptiny", bufs=3, space="PSUM") as ps_tiny,
        ):
            # ---- preload the one ACT table set we use (Sign/Exp/Ln/Copy)
            ld = mybir.InstLoadActFuncSet(
                name=f"I-{nc.next_id()}", ins=[], outs=[],
                act_func_set_id=_act_set_id())
            nc.scalar.add_instruction(ld)

            # ---- consts to SBUF
            csb = {}
            for k, shp in CONST_SHAPES.items():
                t_ = cpool.tile(list(shp), FPR if k in FPR_CONSTS else FP,
                                tag=k)
                nc.sync.dma_start(t_[:], cst[k][:])
                csb[k] = t_

            zcol = cpool.tile([128, 1], FP, tag="zcol")
            nc.vector.memset(zcol[:], 0.0)
            ocol = cpool.tile([128, 1], FP, tag="ocol")
            nc.vector.memset(ocol[:], 1.0)
            m4col = cpool.tile([128, 1], FP, tag="m4col")
            nc.vector.memset(m4col[:], -4.0)
            mcol = cpool.tile([128, 1], FP, tag="mcol")   # ACT bias -1
            nc.vector.memset(mcol[:], -1.0)
            actb = cpool.tile([128, 2], FP, tag="actb")
            nc.vector.memset(actb[:, 0:1], 0.0)
            nc.vector.memset(actb[:, 1:2], 1e-6)

            def bcfill(dst, src2d, *shape):
                nc.vector.tensor_copy(
                    dst, bass.AP(tensor=src2d.tensor, offset=src2d.offset,
                                 ap=[list(src2d.ap[0])]
                                 + [[0, s] for s in shape]))

            # ---- states
            v0a = spool.tile([128, 4096], FPR, tag="v0a")
            v0b = spool.tile([128, 4096], FPR, tag="v0b")
            vna = spool.tile([128, 3072], FPR, tag="vna")
            vnb = spool.tile([128, 3072], FPR, tag="vnb")
            Tt = spool.tile([128, 4], FP, tag="Tt")
            bcfill(v0a[:], zcol[:, 0:1], 4096)
            bcfill(vna[:], zcol[:, 0:1], 3072)
            nc.vector.memset(Tt[:], 0.0)

            # ---- persistent padded out0 tiles, pads preset to -4 once
            o0tiles = []
            for nm in ("o0A", "o0B"):
                o0t = spool.tile([128, 34 * 34], FPR, tag=nm)
                o0v = o0t[:].rearrange("p (h w) -> p h w", h=34)
                bcfill(o0v[:, 0, :], m4col[:, 0:1], 34)
                bcfill(o0v[:, 33, :], m4col[:, 0:1], 34)
                bcfill(o0v[:, 1:33, 0:1], m4col[:, 0:1], 32, 1)
                bcfill(o0v[:, 1:33, 33:34], m4col[:, 0:1], 32, 1)
                o0tiles.append(o0t)

            # ---- x_hi/x_lo -> flat xlin (128-elem zero pad head/tail)
            zrow = cpool.tile([1, 128], FPR, tag="zrow")
            bcfill(zrow[:], zcol[0:1, 0:1], 128)
            for xsrc, xdst, xtag in ((xh, xlinH, "xsb"), (xl, xlinL, "xsb")):
                xsb = cpool.tile([128, 1024], FPR, tag=xtag)
                nc.sync.dma_start(
                    xsb[:],
                    bass.AP(tensor=xsrc, offset=0,
                            ap=[[1024, 128], [1, 1024]]))
                nc.gpsimd.dma_start(
                    bass.AP(tensor=xdst, offset=0,
                            ap=[[128, 1], [1, 128]]),
                    zrow[:])
                nc.gpsimd.dma_start(
                    bass.AP(tensor=xdst, offset=128 + T * 16384,
                            ap=[[128, 1], [1, 128]]),
                    zrow[:])
                nc.gpsimd.dma_start(
                    bass.AP(tensor=xdst, offset=128,
                            ap=[[1024, 128], [1, 1024]]),
                    xsb[:])

            # ---- im2col tiles (row 0 = ones, set once; the rest streamed)
            imA = impool.tile([110, 4096], FPR, tag="imA")
            imB = impool.tile([110, 4096], FPR, tag="imB")
            for imt in (imA, imB):
                bcfill(imt[0:2, :], ocol[0:2, 0:1], 4096)

            def colmat(name, j):
                return csb[name][:, j * 128:(j + 1) * 128]
            ftb2ap = csb["ftb2"][:]

            for t in range(nt):
                v0o, v0n = (v0a, v0b) if t % 2 == 0 else (v0b, v0a)
                vno, vnn = (vna, vnb) if t % 2 == 0 else (vnb, vna)
                im = imA if t % 2 == 0 else imB
                out0p = o0tiles[t % 2]
                o0r = out0p[:].rearrange("p (h w) -> p h w", h=34)

                # ---- im2col DMA: flat 16KB runs, 9 per step
                for dy in range(3):
                    for base, xlin_ in ((2, xlinH), (38, xlinL),
                                        (74, xlinH)):
                        p0 = base + dy * 12
                        nc.sync.dma_start(
                            im[p0:p0 + 12, :],
                            bass.AP(tensor=xlin_,
                                    offset=(128 + t * 16384
                                            + (dy - 1) * 64 - 1),
                                    ap=[[1, 3], [4096, 4], [1, 4096]]))

                # ---- conv0 + LIF0, 8 chunks of 512 (8 h-rows each)
                p1 = wpool.tile([128, 2048], FP, tag="p1")
                for c in range(8):
                    sl = slice(c * 512, (c + 1) * 512)
                    ps = ps_conv.tile([128, 512], FP, tag="pc")
                    Wv = lambda v: csb["w0bd"][:, v * 128:(v + 1) * 128]
                    Wf = lambda v: Wv(v).bitcast(FP)
                    imc = im[:, sl]
                    imf = imc.bitcast(FP)
                    nc.tensor.matmul(ps[:], Wv(0), imc,
                                     start=True, stop=False)
                    nc.tensor.matmul(ps[:, 0::64], Wf(1), imf[:, 0::64],
                                     start=False, stop=False,
                                     skip_group_check=True)
                    nc.tensor.matmul(ps[:, 63::64], Wf(2), imf[:, 63::64],
                                     start=False, stop=False,
                                     skip_group_check=True)
                    if c == 0:
                        nc.tensor.matmul(ps[:, 0:64], Wv(3), imc[:, 0:64],
                                         start=False, stop=False,
                                         skip_group_check=True)
                        nc.tensor.matmul(ps[:, 0:1], Wf(5), imf[:, 0:1],
                                         start=False, stop=False,
                                         skip_group_check=True)
                        nc.tensor.matmul(ps[:, 63:64], Wf(6), imf[:, 63:64],
                                         start=False, stop=False,
                                         skip_group_check=True)
                    if c == 7:
                        nc.tensor.matmul(ps[:, 448:512], Wv(4),
                                         imc[:, 448:512],
                                         start=False, stop=False,
                                         skip_group_check=True)
                        nc.tensor.matmul(ps[:, 448:449], Wf(7),
                                         imf[:, 448:449],
                                         start=False, stop=False,
                                         skip_group_check=True)
                        nc.tensor.matmul(ps[:, 511:512], Wf(8),
                                         imf[:, 511:512],
                                         start=False, stop=False,
                                         skip_group_check=True)
                    nc.tensor.matmul(ps[:], csb["i0"][:], v0o[:, sl],
                                     start=False, stop=True,
                                     skip_group_check=True)
                    # s' = Sign(u-1) on ACT ; v' = (s'<0)*u on DVE
                    s0c = wpool.tile([128, 512], FP, tag="s0c")
                    nc.scalar.activation(s0c[:], ps[:], Act.Sign,
                                         bias=mcol[:, 0:1], scale=1.0)
                    nc.vector.scalar_tensor_tensor(
                        v0n[:, sl], s0c[:], 0.0, ps[:], Alu.is_lt, Alu.mult)
                    s0r = s0c[:].rearrange("p (h w) -> p h w", h=8)
                    p1r = p1[:].rearrange("p (h w) -> p h w", h=64)
                    nc.vector.tensor_tensor(
                        p1r[:, c * 8:(c + 1) * 8, :],
                        s0r[:, :, 0::2], s0r[:, :, 1::2], Alu.add)

                # ---- pool rows into padded out0 (S in [-4,4]) + f0 sum
                f0sum = tpool.tile([128, 1], FP, tag="f0sum")
                p1v = p1[:].rearrange("p (h w) -> p h w", h=64)
                nc.vector.tensor_tensor(
                    o0r[:, 1:33, 1:33], p1v[:, 0::2, :], p1v[:, 1::2, :],
                    Alu.add)
                nc.vector.tensor_reduce(f0sum[:], o0r[:, 1:33, 1:33],
                                        mybir.AxisListType.XY, Alu.add)

                # ---- f04 = 0.4*relu(ft @ mean + ftb)   (folded consts)
                psf0 = ps_tiny.tile([128, 1], FP, tag="gt")
                nc.tensor.matmul(psf0[:], colmat("ftmm", 0), f0sum[:],
                                 start=True, stop=True)
                f04 = tpool.tile([128, 1], FP, tag="f04")
                nc.vector.tensor_scalar(f04[:], psf0[:], ftb2ap, 0.0,
                                        Alu.add, op1=Alu.max)

                # ---- trace row0 pre-update
                nc.vector.scalar_tensor_tensor(
                    Tt[:, 0:1], Tt[:, 0:1], DECAY, f04[:], Alu.mult, Alu.add)

                # ================= graph math =================
                def tiny(tag, p_, f_, dt_=FP):
                    return tpool.tile([p_, f_], dt_, tag=tag, name=tag)

                psg = ps_tiny.tile([128, 4], FP, tag="gt")
                nc.tensor.matmul(psg[:], csb["gwbd"][:], Tt[:],
                                 start=True, stop=True)
                hpc = tiny("hpc", 128, 4)
                nc.vector.tensor_copy(hpc[:], psg[:])

                pse1 = ps_tiny.tile([8, 4], FP, tag="gt")
                nc.tensor.matmul(pse1[:], csb["ga1"][:], hpc[:],
                                 start=True, stop=True)
                e1t = tiny("e1t", 8, 4)
                nc.vector.tensor_copy(e1t[:], pse1[:])
                pse2 = ps_tiny.tile([8, 4], FP, tag="gt")
                nc.tensor.matmul(pse2[:], csb["ga2"][:], hpc[:],
                                 start=True, stop=True)
                e2t = tiny("e2t", 8, 4)
                nc.vector.tensor_copy(e2t[:], pse2[:])

                def reap(ap_, tail):
                    dims = [list(d) for d in ap_.ap][:-1] + tail
                    return bass.AP(tensor=ap_.tensor, offset=ap_.offset,
                                   ap=dims)

                def bc_n(ap_):  # [p,4] -> free (n,m): n varies, m bcast
                    return reap(ap_, [[1, 4], [0, 4]])

                def bc_m(ap_):  # free (n,m): n bcast, m varies
                    return reap(ap_, [[0, 4], [1, 4]])

                es = tiny("es", 8, 16)
                nc.vector.tensor_tensor(es[:], bc_n(e1t[:]), bc_m(e2t[:]),
                                        Alu.add)
                es2 = tiny("es2", 8, 16)
                nc.vector.tensor_scalar_mul(es2[:], es[:], 0.2)
                el = tiny("el", 8, 16)
                nc.vector.tensor_tensor(el[:], es[:], es2[:], Alu.max)

                psE = ps_tiny.tile([2, 16], FP, tag="gt")
                nc.tensor.matmul(psE[:], csb["ghbd"][:], el[:],
                                 start=True, stop=True)
                Ec = tiny("Ec", 2, 16)
                nc.vector.tensor_copy(Ec[:], psE[:])

                def tr_nm(ap_):  # read transposed over (n,m)
                    return reap(ap_, [[1, 4], [4, 4]])

                L = tiny("L", 2, 16)
                nc.vector.tensor_tensor(L[:], Ec[:], tr_nm(Ec[:]), Alu.add)
                Lr = L[:].rearrange("p (n m) -> p n m", n=4)
                mx = tiny("mx", 2, 4)
                nc.vector.tensor_reduce(mx[:], Lr, mybir.AxisListType.X,
                                        Alu.max)
                xm = tiny("xm", 2, 16)
                nc.vector.tensor_tensor(xm[:], L[:], bc_n(mx[:]), Alu.subtract)
                ex = tiny("ex", 2, 16)
                nc.scalar.activation(ex[:], xm[:], Act.Exp,
                                     bias=actb[0:2, 0:1])
                sm = tiny("sm", 2, 4)
                exr = ex[:].rearrange("p (n m) -> p n m", n=4)
                nc.vector.tensor_reduce(sm[:], exr, mybir.AxisListType.X,
                                        Alu.add)
                rc = tiny("rc", 2, 4)
                nc.vector.reciprocal(rc[:], sm[:])
                S = tiny("S", 2, 16)
                nc.vector.tensor_tensor(S[:], ex[:], bc_n(rc[:]), Alu.mult)

                Sr = S[:].rearrange("p (n m) -> p n m", n=4)
                lo = tiny("lo", 2, 8)
                lor = lo[:].rearrange("p (n m) -> p n m", n=4)
                hi = tiny("hi", 2, 8)
                hir = hi[:].rearrange("p (n m) -> p n m", n=4)
                nc.vector.tensor_tensor(lor, Sr[:, :, 0::2], Sr[:, :, 1::2],
                                        Alu.min)
                nc.vector.tensor_tensor(hir, Sr[:, :, 0::2], Sr[:, :, 1::2],
                                        Alu.max)
                kth = tiny("kth", 2, 4)
                l2 = tiny("l2", 2, 4)
                nc.vector.tensor_tensor(l2[:], lor[:, :, 0], lor[:, :, 1],
                                        Alu.max)
                h2 = tiny("h2", 2, 4)
                nc.vector.tensor_tensor(h2[:], hir[:, :, 0], hir[:, :, 1],
                                        Alu.min)
                nc.vector.tensor_tensor(kth[:], l2[:], h2[:], Alu.min)
                msk = tiny("msk", 2, 16)
                nc.vector.tensor_tensor(msk[:], S[:], bc_n(kth[:]), Alu.is_ge)
                Sp = tiny("Sp", 2, 16)
                nc.vector.tensor_tensor(Sp[:], S[:], msk[:], Alu.mult)

                A2 = tiny("A2", 2, 16)
                nc.vector.tensor_tensor(A2[:], Sp[:], tr_nm(Sp[:]), Alu.add)
                rs = tiny("rs", 2, 4)
                A2r = A2[:].rearrange("p (n m) -> p n m", n=4)
                nc.vector.tensor_reduce(rs[:], A2r, mybir.AxisListType.X,
                                        Alu.add)
                lnd = tiny("lnd", 2, 4)
                nc.scalar.activation(lnd[:], rs[:], Act.Ln,
                                     bias=actb[0:2, 1:2], scale=0.5)
                q = tiny("q", 2, 4)
                nc.scalar.activation(q[:], lnd[:], Act.Exp, scale=-0.5,
                                     bias=actb[0:2, 0:1])

                t1 = tiny("t1", 2, 16)
                nc.vector.tensor_tensor(t1[:], A2[:], bc_n(q[:]), Alu.mult)
                OPt = tiny("OPt", 2, 16)
                nc.vector.scalar_tensor_tensor(OPt[:], t1[:], 0.5, bc_m(q[:]),
                                               Alu.mult, Alu.mult)
                col0 = reap(OPt[:], [[0, 4], [4, 4]])
                t2 = tiny("t2", 2, 16)
                nc.vector.tensor_tensor(t2[:], OPt[:], col0, Alu.mult)
                af = tiny("af", 2, 4)
                t2r = t2[:].rearrange("p (n m) -> p n m", n=4)
                nc.vector.tensor_reduce(af[:], t2r, mybir.AxisListType.X,
                                        Alu.add)
                # al3f [2,4] fpr: cols 1-3 = alpha*cn, col 0 garbage*0=0
                al3f = tiny("al3f", 2, 4, FPR)
                nc.vector.tensor_tensor(al3f[:], af[:], csb["cnrow4"][:],
                                        Alu.mult)
                psb = ps_tiny.tile([128, 4], FP, tag="gt")
                nc.tensor.matmul(psb[:], csb["gbc"][:], al3f[:],
                                 start=True, stop=True)
                aap = tiny("aap", 128, 4)
                nc.vector.tensor_copy(aap[:], psb[:])

                # ================= node path =================
                sn = wpool.tile([128, 3072], FP, tag="sn")
                snsum = tpool.tile([128, 3], FP, tag="snsum")
                snsumB = tpool.tile([128, 3], FP, tag="snsumB")
                sw = [swpool.tile([128, 9 * 128], FPR, tag=f"sw{n}",
                                  name=f"sw{n}") for n in range(3)]
                for n in range(3):
                    nc.vector.tensor_scalar_mul(
                        sw[n][:],
                        csb["wnod"][:, n * 9 * 128:(n + 1) * 9 * 128],
                        aap[:, n + 1:n + 2])
                for n in range(3):
                    for c in range(2):
                        psn = ps_node.tile([128, 512], FP, tag="pn")
                        for k in range(9):
                            dy, dx = k // 3, k % 3
                            rhs = o0r[:, dy + 16 * c: dy + 16 * c + 16,
                                      dx:dx + 32]
                            nc.tensor.matmul(psn[:],
                                             sw[n][:, k * 128:(k + 1) * 128],
                                             rhs, start=(k == 0), stop=False)
                        nc.tensor.matmul(
                            psn[:], csb["biasn"][0:1, n * 128:(n + 1) * 128],
                            im[0:1, c * 512:(c + 1) * 512],
                            start=False, stop=False)
                        nc.tensor.matmul(
                            psn[:], csb["apad"][:, n * 128:(n + 1) * 128],
                            reap(al3f[:, n + 1:n + 2], [[0, 512]]),
                            start=False, stop=False, skip_group_check=True)
                        nc.tensor.matmul(
                            psn[:], colmat("in3", n),
                            vno[:, n * 1024 + c * 512:
                                n * 1024 + (c + 1) * 512],
                            start=False, stop=True)
                        sl = slice(n * 1024 + c * 512,
                                   n * 1024 + (c + 1) * 512)
                        nc.scalar.activation(
                            sn[:, sl], psn[:], Act.Sign, bias=mcol[:, 0:1],
                            accum_out=(snsum if c == 0
                                       else snsumB)[:, n:n + 1])
                        nc.vector.scalar_tensor_tensor(
                            vnn[:, sl], sn[:, sl], 0.0, psn[:],
                            Alu.is_lt, Alu.mult)

                # ---- feats + trace update
                psf = ps_tiny.tile([128, 3], FP, tag="gt")
                nc.tensor.matmul(psf[:], colmat("ftmm", 1), snsum[:],
                                 start=True, stop=False)
                nc.tensor.matmul(psf[:], colmat("ftmm", 1), snsumB[:],
                                 start=False, stop=True)
                fn04 = tpool.tile([128, 3], FP, tag="fn04")
                nc.vector.tensor_scalar(fn04[:], psf[:], ftb2ap, 0.0,
                                        Alu.add, op1=Alu.max)
                nc.vector.scalar_tensor_tensor(
                    Tt[:, 0:1], Tt[:, 0:1], DECAY, f04[:], Alu.mult, Alu.add)
                nc.vector.scalar_tensor_tensor(
                    Tt[:, 1:4], Tt[:, 1:4], DECAY, fn04[:], Alu.mult, Alu.add)

                # ================= output y (DVE) =================
                # y = yw0*S0 + sum_n ywn*sn' + yc
                ysb = wpool.tile([128, 1024], FP, tag="ysb")
                nc.vector.tensor_scalar(ysb[:], o0r[:, 1:33, 1:33],
                                        yw[0], yc, Alu.mult, op1=Alu.add)
                for n in range(3):
                    nc.vector.scalar_tensor_tensor(
                        ysb[:], sn[:, n * 1024:(n + 1) * 1024], yw[n + 1],
                        ysb[:], Alu.mult, Alu.add)
                nc.gpsimd.dma_start(
                    bass.AP(tensor=y, offset=t * BC * CO * 1024,
                            ap=[[1024, 128], [1, 1024]]),
                    ysb[:])
    if not nc.is_finalized():
        nc.finalize()
    return nc


_NC_CACHE = {}


def _get_nc(nt=T, yw=(0.125, 0.5, 0.5, 0.5), yc=1.0):
    key = (nt, tuple(float(v) for v in yw), float(yc))
    if key not in _NC_CACHE:
        _NC_CACHE[key] = build_nc(nt, yw, yc)
    return _NC_CACHE[key]


def _split_hi_lo(x):
    """Split fp32 array into hi (11 mantissa bits, fp32r-exact) + lo."""
    xb = np.ascontiguousarray(x, np.float32)
    hi = (xb.view(np.uint32) & np.uint32(0xFFFFF000)).view(np.float32)
    lo = (xb - hi).astype(np.float32)
    return hi, lo


def kernel(**inputs):
    x = np.asarray(inputs["x"], np.float32)
    consts = _host_consts(
        inputs["conv0_w"], inputs["bn0_g"], inputs["bn0_b"], inputs["bn0_m"],
        inputs["bn0_v"], inputs["lif0_w"], inputs["convs_w"], inputs["bns_g"],
        inputs["bns_b"], inputs["bns_m"], inputs["bns_v"], inputs["lifs_w"],
        inputs["ft_w"], inputs["ft_b"], inputs["gat_w"], inputs["gat_a"],
        inputs["out_weights"])
    consts = {k: np.ascontiguousarray(v, np.float32)
              for k, v in consts.items()}
    sigw = 1.0 / (1.0 + np.exp(-np.asarray(inputs["out_weights"], np.float64)))
    yw = (float(sigw[0]) / 8.0, float(sigw[1]) / 2.0, float(sigw[2]) / 2.0,
          float(sigw[3]) / 2.0)
    yc = float(sigw[0] / 2.0 + (sigw[1] + sigw[2] + sigw[3]) / 2.0)
    nc = _get_nc(T, yw, yc)
    xhi, xlo = _split_hi_lo(x)
    core_ids = list(range(NCORES))
    in_maps = []
    for k in core_ids:
        m = dict(consts)
        m["xh"] = np.ascontiguousarray(xhi[:, k * BC:(k + 1) * BC])
        m["xl"] = np.ascontiguousarray(xlo[:, k * BC:(k + 1) * BC])
        in_maps.append(m)
    res = run_bass_kernel_spmd(nc, in_maps, core_ids).results
    out = np.concatenate([res[k]["y"] for k in core_ids], axis=1)
    return out.astype(np.float32)


# revision 40
# speedup vs baseline: 1.6430x; 1.1080x over previous
"""STSPBlock Trainium2 kernel.

Structure (per core, batch-sharded B=16 -> 8 cores x B=2):
  partitions p = b*64 + channel for all activation tensors.

  - conv0+bn+LIF-input-scale folded into one K=73 im2col matmul.
    x is split HOST-side into hi (11 mantissa bits, exactly
    representable under the PE's float32r moving-operand rounding) and
    lo (residual): 36 hi tap rows + 36 lo tap rows + 1 ones row
    (bias), so the fp32r matmul is numerically exact for the conv.
    Taps are DMA'd from DRAM copies of x_hi/x_lo laid out as
    zero-PADDED 66x66 planes, so no edge-correction matmuls are
    needed. The LIF state add (1-c0)*v rides the same PSUM group via a
    scaled-identity fp32r matmul.

  - Spikes are computed on the SCALAR engine as s' = Sign(u - 1) in
    {-1,+1} ("sign encoding", s = (s'+1)/2); every consumer is linear
    in s, so the affine fix is folded host-side into the next conv's
    weights/bias, the feat-transform consts, and the y weights. The
    out0 pool pad cells are preset to -4 (the sign encoding of 0).
    Sign/Exp/Ln/Copy all live in one ACT table set
    (natural_log_exp_and_others), preloaded once.

  - reset v' = (s' < 0) * u on DVE; avgpool via paired adds (values
    become integer sums in [-4,4] -> exact under fp32r).

  - BETA=0 => S-state is just alpha each step. alpha is applied by
    scaling the block-diag node conv weights; the sign-encoding
    constant term (alpha-dependent, spatially uniform thanks to the -4
    padding) enters PSUM via a tiny K=2 matmul whose rhs is alpha
    broadcast with a 0-stride AP.

  - y = affine combination of sign spikes on DVE.

All bn/LIF/sigmoid parameter folding is done host-side from the actual
input values at call time, so the kernel is fully general.
"""

import numpy as np

import concourse.bass as bass
import concourse.bacc as bacc
import concourse.mybir as mybir
from concourse.tile import TileContext
from concourse.bass_utils import run_bass_kernel_spmd

FP = mybir.dt.float32
FPR = mybir.dt.float32r
Alu = mybir.AluOpType
Act = mybir.ActivationFunctionType

T, BFULL, CIN, H, W = 8, 16, 2, 64, 64
CO, NN, HEADS = 64, 4, 4
HP, WP = 32, 32
BC = 2                    # batch per core
NCORES = 8
EPS = 1e-5
DECAY = 0.6
HD = CO // HEADS          # 16
PL = 66 * 66              # padded plane size
NPL = 4                   # planes per timestep (b, ci)

ACT_SET_NLE = None  # index of the Sign+Exp+Ln+Copy ACT table set


# ----------------------------------------------------------------- host consts
def _host_consts(conv0_w, bn0_g, bn0_b, bn0_m, bn0_v, lif0_w,
                 convs_w, bns_g, bns_b, bns_m, bns_v, lifs_w,
                 ft_w, ft_b, gat_w, gat_a, out_weights):
    f32 = np.float32
    sig = lambda z: 1.0 / (1.0 + np.exp(-z.astype(np.float64)))
    c0 = f32(sig(lif0_w))
    cn = sig(lifs_w).astype(f32)          # [3]

    s0c = (bn0_g / np.sqrt(bn0_v + EPS)).astype(f32)
    bias0 = ((bn0_b - bn0_m * s0c) * c0).astype(f32)
    W0f = (conv0_w * s0c[:, None, None, None] * c0).astype(f32)  # [64,2,3,3]

    # fp32r rounds BOTH matmul operands to ~12 mantissa bits. Split
    # weights and x into hi (11-bit, exactly representable) + lo parts
    # and keep the three cross products Whi*xhi + Whi*xlo + Wlo*xhi
    # (dropped Wlo*xlo term is ~2^-22 relative): the conv is exact.
    def tr11(a):
        return (np.ascontiguousarray(a, f32).view(np.uint32)
                & np.uint32(0xFFFFF000)).view(f32)

    W0hi = tr11(W0f)
    W0lo = (W0f - W0hi).astype(f32)
    b0hi = tr11(bias0)
    b0lo = (bias0 - b0hi).astype(f32)

    # w0bd [110, 3*128]: rows 0/1 = bias hi/lo (ones taps); rows 2-37 =
    # Whi (vs x_hi rows), 38-73 = Whi (vs x_lo), 74-109 = Wlo (vs x_hi
    # again). col m = v*128 + b*64 + co. x rows are host-padded to 66
    # wide with zero pad columns, so horizontal tap overflow reads
    # zeros; only vertical overflow needs fixing: variant 1 subtracts
    # the dy=0 taps' garbage at h=0, variant 2 the dy=2 taps' at h=63.
    w0bd = np.zeros((110, 3 * 128), f32)
    w0bd[0, 0:64] = b0hi
    w0bd[0, 64:128] = b0hi
    w0bd[1, 0:64] = b0lo
    w0bd[1, 64:128] = b0lo

    def put(v, dy, dx, sgn):
        for b in range(2):
            for ci in range(2):
                p = dy * 12 + dx * 4 + b * 2 + ci
                c0_, c1_ = v * 128 + b * 64, v * 128 + (b + 1) * 64
                w0bd[2 + p, c0_:c1_] = sgn * W0hi[:, ci, dy, dx]
                w0bd[38 + p, c0_:c1_] = sgn * W0hi[:, ci, dy, dx]
                w0bd[74 + p, c0_:c1_] = sgn * W0lo[:, ci, dy, dx]

    for dy in range(3):
        for dx in range(3):
            put(0, dy, dx, 1.0)
    for dx in range(3):
        put(1, 0, dx, -1.0)
        put(2, 2, dx, -1.0)

    i0 = ((1.0 - c0) * np.eye(128)).astype(f32)

    sncol = (bns_g / np.sqrt(bns_v + EPS)).astype(f32)            # [3,64]
    biasn_raw = (bns_b - bns_m * sncol).astype(f32)               # [3,64]
    # 0.25 = avgpool fold; extra 0.5 = sign-encoding decode s=(s'+1)/2
    Wf = (convs_w * sncol[:, :, None, None, None] * 0.25).astype(f32)
    Wh = (Wf * 0.5).astype(f32)

    # wnod [3, 9, 128, 128]: per (node, tap) block-diag lhsT over b
    wnod = np.zeros((3, 9, 128, 128), f32)
    for n in range(3):
        for dy in range(3):
            for dx in range(3):
                k = dy * 3 + dx
                blk = Wh[n, :, :, dy, dx].T    # [ci, co]
                wnod[n, k, 0:64, 0:64] = blk
                wnod[n, k, 64:128, 64:128] = blk

    in3 = np.stack([(1.0 - cn[n]) * np.eye(128) for n in range(3)]).astype(f32)
    biasn = np.concatenate([np.tile(cn[n] * biasn_raw[n], 2)
                            for n in range(3)]).reshape(1, 384).astype(f32)

    # apad [2, 3*128]: sign-encoding uniform term 2*sum_taps(Wf)[n,co];
    # rhs = alpha*cn broadcast, so the term becomes alpha*cn*2*tapsum.
    tapsum = Wf.sum(axis=(2, 3, 4))            # [3, 64]
    apad = np.zeros((2, 3 * 128), f32)
    for n in range(3):
        for b in range(2):
            apad[b, n * 128 + b * 64:n * 128 + (b + 1) * 64] = 2.0 * tapsum[n]

    def bd(m):  # block-diag [128,128] of m.T twice ([co,ci] -> lhsT)
        z = np.zeros((128, 128), f32)
        z[0:64, 0:64] = m.T
        z[64:128, 64:128] = m.T
        return z

    # feat transform: f04 = 0.4*relu(ftw @ mean + ftb), sign-decode and
    # the 0.4 trace factor folded:  mean0 = (0.125/1024)*S0sum + 0.5,
    # meann = (0.5/1024)*Snsum + 0.5.
    ftmm = np.stack([bd(ft_w * (0.4 * 0.125 / 1024.0)),
                     bd(ft_w * (0.4 * 0.5 / 1024.0))])
    ftb_f = (0.4 * (ft_b + 0.5 * ft_w.sum(axis=1))).astype(f32)
    ftb2 = np.tile(ftb_f, 2).reshape(128, 1).astype(f32)
    gwbd = bd(gat_w).astype(f32)

    # ga1/ga2 [128, 8]: in p=(b, c') c'=h*16+d ; out m = b*4+h
    ga1 = np.zeros((128, 8), f32)
    ga2 = np.zeros((128, 8), f32)
    for b in range(2):
        for h in range(HEADS):
            for d in range(HD):
                ga1[b * 64 + h * 16 + d, b * 4 + h] = gat_a[h, d]
                ga2[b * 64 + h * 16 + d, b * 4 + h] = gat_a[h, HD + d]

    # ghbd [8,2]: p=(b,h) -> col b ; carries 0.5(sym)*0.25(mean h)/0.01(temp)
    ghbd = np.zeros((8, 2), f32)
    for b in range(2):
        ghbd[b * 4:(b + 1) * 4, b] = 12.5

    gbc = np.zeros((2, 128), f32)
    gbc[0, 0:64] = 1.0
    gbc[1, 64:128] = 1.0

    # cnrow4 [2,4]: col 0 unused (node 0 has no conv), cols 1-3 = cn
    cnrow4 = np.zeros((2, 4), f32)
    cnrow4[:, 1:4] = cn[None, :]

    def cols(stk):  # [k,128,128] -> [128, k*128]
        return np.ascontiguousarray(
            np.transpose(stk, (1, 0, 2)).reshape(128, -1))

    return dict(w0bd=w0bd, i0=i0, wnod=cols(wnod.reshape(27, 128, 128)),
                in3=cols(in3), biasn=biasn, apad=apad,
                ftmm=cols(ftmm), ftb2=ftb2, gwbd=gwbd,
                ga1=ga1, ga2=ga2, ghbd=ghbd, gbc=gbc, cnrow4=cnrow4)


CONST_SHAPES = dict(w0bd=(110, 3 * 128), i0=(128, 128), wnod=(128, 27 * 128),
                    in3=(128, 3 * 128), biasn=(1, 384), apad=(2, 3 * 128),
                    ftmm=(128, 2 * 128), ftb2=(128, 1), gwbd=(128, 128),
                    ga1=(128, 8), ga2=(128, 8), ghbd=(8, 2), gbc=(2, 128),
                    cnrow4=(2, 4))
# consts that feed the big float32r matmuls
FPR_CONSTS = {"w0bd", "i0", "wnod", "in3", "biasn", "apad", "gbc"}


def _act_set_id():
    global ACT_SET_NLE
    if ACT_SET_NLE is None:
        from concourse.hw_specs import get_activation_tables
        for i, name in enumerate(get_activation_tables("gen3")):
            if name == "natural_log_exp_and_others":
                ACT_SET_NLE = i
                break
        assert ACT_SET_NLE is not None
    return ACT_SET_NLE


# ------------------------------------------------------------------ the module
def build_nc(nt=T, yw=(0.125, 0.5, 0.5, 0.5), yc=1.0):
    nc = bacc.Bacc(None, target_bir_lowering=False)
    xh = nc.declare_dram_parameter("xh", [T, BC, CIN, H, 66], FPR,
                                   isOutput=False)
    xl = nc.declare_dram_parameter("xl", [T, BC, CIN, H, 66], FPR,
                                   isOutput=False)
    cst = {k: nc.declare_dram_parameter(
               k, list(v), FPR if k in FPR_CONSTS else FP, isOutput=False)
           for k, v in CONST_SHAPES.items()}
    y = nc.declare_dram_parameter("y", [T, BC, CO, HP, WP], FP, isOutput=True)
    xlinH = nc.dram_tensor("xlinH", [T * 16896 + 256], FPR)
    xlinL = nc.dram_tensor("xlinL", [T * 16896 + 256], FPR)

    with TileContext(nc) as tc:
        with (
            tc.tile_pool(name="consts", bufs=1) as cpool,
            tc.tile_pool(name="state", bufs=1) as spool,
            tc.tile_pool(name="im", bufs=1) as impool,
            tc.tile_pool(name="work", bufs=2) as wpool,
            tc.tile_pool(name="sw", bufs=1) as swpool,
            tc.tile_pool(name="tiny", bufs=3) as tpool,
            tc.tile_pool(name="pconv", bufs=2, space="PSUM") as ps_conv,
            tc.tile_pool(name="pnode", bufs=2, space="PSUM") as ps_node,
            tc.tile_pool(name="ptiny", bufs=3, space="PSUM") as ps_tiny,
        ):
            # ---- preload the one ACT table set we use (Sign/Exp/Ln/Copy)
            ld = mybir.InstLoadActFuncSet(
                name=f"I-{nc.next_id()}", ins=[], outs=[],
                act_func_set_id=_act_set_id())
            nc.scalar.add_instruction(ld)

            # ---- consts to SBUF
            csb = {}
            for k, shp in CONST_SHAPES.items():
                t_ = cpool.tile(list(shp), FPR if k in FPR_CONSTS else FP,
                                tag=k)
                nc.sync.dma_start(t_[:], cst[k][:])
                csb[k] = t_

            zcol = cpool.tile([128, 1], FP, tag="zcol")
            nc.vector.memset(zcol[:], 0.0)
            ocol = cpool.tile([128, 1], FP, tag="ocol")
            nc.vector.memset(ocol[:], 1.0)
            m4col = cpool.tile([128, 1], FP, tag="m4col")
            nc.vector.memset(m4col[:], -4.0)
            mcol = cpool.tile([128, 1], FP, tag="mcol")   # ACT bias -1
            nc.vector.memset(mcol[:], -1.0)
            actb = cpool.tile([128, 2], FP, tag="actb")
            nc.vector.memset(actb[:, 0:1], 0.0)
            nc.vector.memset(actb[:, 1:2], 1e-6)

            def bcfill(dst, src2d, *shape):
                nc.vector.tensor_copy(
                    dst, bass.AP(tensor=src2d.tensor, offset=src2d.offset,
                                 ap=[list(src2d.ap[0])]
                                 + [[0, s] for s in shape]))

            # ---- states
            v0a = spool.tile([128, 4096], FPR, tag="v0a")
            v0b = spool.tile([128, 4096], FPR, tag="v0b")
            vna = spool.tile([128, 3072], FPR, tag="vna")
            vnb = spool.tile([128, 3072], FPR, tag="vnb")
            Tt = spool.tile([128, 4], FP, tag="Tt")
            bcfill(v0a[:], zcol[:, 0:1], 4096)
            bcfill(vna[:], zcol[:, 0:1], 3072)
            nc.vector.memset(Tt[:], 0.0)

            # ---- persistent padded out0 tiles, pads preset to -4 once
            o0tiles = []
            for nm in ("o0A", "o0B"):
                o0t = spool.tile([128, 34 * 34], FPR, tag=nm)
                o0v = o0t[:].rearrange("p (h w) -> p h w", h=34)
                bcfill(o0v[:, 0, :], m4col[:, 0:1], 34)
                bcfill(o0v[:, 33, :], m4col[:, 0:1], 34)
                bcfill(o0v[:, 1:33, 0:1], m4col[:, 0:1], 32, 1)
                bcfill(o0v[:, 1:33, 33:34], m4col[:, 0:1], 32, 1)
                o0tiles.append(o0t)

            # ---- x_hi/x_lo -> flat xlin (128-elem zero pad head/tail)
            zrow = cpool.tile([1, 128], FPR, tag="zrow")
            bcfill(zrow[:], zcol[0:1, 0:1], 128)
            for xsrc, xdst, xtag in ((xh, xlinH, "xsb"), (xl, xlinL, "xsb")):
                xsb = cpool.tile([128, 1056], FPR, tag=xtag)
                nc.sync.dma_start(
                    xsb[:],
                    bass.AP(tensor=xsrc, offset=0,
                            ap=[[1056, 128], [1, 1056]]))
                nc.gpsimd.dma_start(
                    bass.AP(tensor=xdst, offset=0,
                            ap=[[128, 1], [1, 128]]),
                    zrow[:])
                nc.gpsimd.dma_start(
                    bass.AP(tensor=xdst, offset=128 + T * 16896,
                            ap=[[128, 1], [1, 128]]),
                    zrow[:])
                nc.gpsimd.dma_start(
                    bass.AP(tensor=xdst, offset=128,
                            ap=[[1056, 128], [1, 1056]]),
                    xsb[:])

            # ---- im2col tiles (row 0 = ones, set once; the rest streamed)
            imA = impool.tile([110, 4224], FPR, tag="imA")
            imB = impool.tile([110, 4224], FPR, tag="imB")
            for imt in (imA, imB):
                bcfill(imt[0:2, :], ocol[0:2, 0:1], 4224)

            def colmat(name, j):
                return csb[name][:, j * 128:(j + 1) * 128]
            ftb2ap = csb["ftb2"][:]

            def tiny(tag, p_, f_, dt_=FP):
                return tpool.tile([p_, f_], dt_, tag=tag, name=tag)

            def reap(ap_, tail):
                dims = [list(d) for d in ap_.ap][:-1] + tail
                return bass.AP(tensor=ap_.tensor, offset=ap_.offset,
                               ap=dims)

            def bc_n(ap_):  # [p,4] -> free (n,m): n varies, m bcast
                return reap(ap_, [[1, 4], [0, 4]])

            def bc_m(ap_):  # free (n,m): n bcast, m varies
                return reap(ap_, [[0, 4], [1, 4]])

            def tr_nm(ap_):  # read transposed over (n,m)
                return reap(ap_, [[1, 4], [4, 4]])

            def im2col(t):
                im = imA if t % 2 == 0 else imB
                for dy in range(3):
                    for base, xlin_ in ((2, xlinH), (38, xlinL),
                                        (74, xlinH)):
                        p0 = base + dy * 12
                        nc.sync.dma_start(
                            im[p0:p0 + 12, :],
                            bass.AP(tensor=xlin_,
                                    offset=(128 + t * 16896
                                            + (dy - 1) * 66 - 1),
                                    ap=[[1, 3], [4224, 4], [1, 4224]]))

            # per-step tiles handed from graph(t) to node(t)
            hand = {}

            def conv_block(t):
                v0o, v0n = (v0a, v0b) if t % 2 == 0 else (v0b, v0a)
                im = imA if t % 2 == 0 else imB
                imv = im[:].rearrange("p (h w) -> p h w", h=64)
                o0r = o0tiles[t % 2][:].rearrange("p (h w) -> p h w", h=34)
                p1 = wpool.tile([128, 2048], FP, tag="p1")
                Wv = lambda v: csb["w0bd"][:, v * 128:(v + 1) * 128]
                for c in range(8):
                    sl = slice(c * 512, (c + 1) * 512)
                    ps = ps_conv.tile([128, 512], FP, tag="pc")
                    nc.tensor.matmul(ps[:], Wv(0),
                                     imv[:, c * 8:(c + 1) * 8, 0:64],
                                     start=True, stop=False)
                    if c == 0:
                        # subtract dy=0 taps' vertical-overflow garbage
                        nc.tensor.matmul(ps[:, 0:64], Wv(1),
                                         imv[:, 0:1, 0:64],
                                         start=False, stop=False,
                                         skip_group_check=True)
                    if c == 7:
                        nc.tensor.matmul(ps[:, 448:512], Wv(2),
                                         imv[:, 63:64, 0:64],
                                         start=False, stop=False,
                                         skip_group_check=True)
                    nc.tensor.matmul(ps[:], csb["i0"][:], v0o[:, sl],
                                     start=False, stop=True,
                                     skip_group_check=True)
                    # s' = Sign(u-1) on ACT ; v' = (s'<0)*u on DVE
                    s0c = wpool.tile([128, 512], FP, tag="s0c")
                    nc.scalar.activation(s0c[:], ps[:], Act.Sign,
                                         bias=mcol[:, 0:1], scale=1.0)
                    nc.vector.scalar_tensor_tensor(
                        v0n[:, sl], s0c[:], 0.0, ps[:], Alu.is_lt, Alu.mult)
                    s0r = s0c[:].rearrange("p (h w) -> p h w", h=8)
                    p1r = p1[:].rearrange("p (h w) -> p h w", h=64)
                    nc.vector.tensor_tensor(
                        p1r[:, c * 8:(c + 1) * 8, :],
                        s0r[:, :, 0::2], s0r[:, :, 1::2], Alu.add)

                # pool rows into padded out0 (S in [-4,4]) + f0 sum
                f0sum = tiny("f0sum", 128, 1)
                p1v = p1[:].rearrange("p (h w) -> p h w", h=64)
                nc.vector.tensor_tensor(
                    o0r[:, 1:33, 1:33], p1v[:, 0::2, :], p1v[:, 1::2, :],
                    Alu.add)
                nc.vector.tensor_reduce(f0sum[:], o0r[:, 1:33, 1:33],
                                        mybir.AxisListType.XY, Alu.add)
                hand[("f0sum", t)] = f0sum

            def graph_block(t):
                f0sum = hand.pop(("f0sum", t))
                psf0 = ps_tiny.tile([128, 1], FP, tag="gt")
                nc.tensor.matmul(psf0[:], colmat("ftmm", 0), f0sum[:],
                                 start=True, stop=True)
                f04 = tiny("f04", 128, 1)
                nc.vector.tensor_scalar(f04[:], psf0[:], ftb2ap, 0.0,
                                        Alu.add, op1=Alu.max)
                hand[("f04", t)] = f04
                # trace row0 pre-update
                nc.vector.scalar_tensor_tensor(
                    Tt[:, 0:1], Tt[:, 0:1], DECAY, f04[:], Alu.mult, Alu.add)

                psg = ps_tiny.tile([128, 4], FP, tag="gt")
                nc.tensor.matmul(psg[:], csb["gwbd"][:], Tt[:],
                                 start=True, stop=True)
                hpc = tiny("hpc", 128, 4)
                nc.vector.tensor_copy(hpc[:], psg[:])

                pse1 = ps_tiny.tile([8, 4], FP, tag="gt")
                nc.tensor.matmul(pse1[:], csb["ga1"][:], hpc[:],
                                 start=True, stop=True)
                e1t = tiny("e1t", 8, 4)
                nc.vector.tensor_copy(e1t[:], pse1[:])
                pse2 = ps_tiny.tile([8, 4], FP, tag="gt")
                nc.tensor.matmul(pse2[:], csb["ga2"][:], hpc[:],
                                 start=True, stop=True)
                e2t = tiny("e2t", 8, 4)
                nc.vector.tensor_copy(e2t[:], pse2[:])

                es = tiny("es", 8, 16)
                nc.vector.tensor_tensor(es[:], bc_n(e1t[:]), bc_m(e2t[:]),
                                        Alu.add)
                el = tiny("el", 8, 16)
                nc.vector.scalar_tensor_tensor(el[:], es[:], 0.2, es[:],
                                               Alu.mult, Alu.max)

                psE = ps_tiny.tile([2, 16], FP, tag="gt")
                nc.tensor.matmul(psE[:], csb["ghbd"][:], el[:],
                                 start=True, stop=True)
                Ec = tiny("Ec", 2, 16)
                nc.vector.tensor_copy(Ec[:], psE[:])

                L = tiny("L", 2, 16)
                nc.vector.tensor_tensor(L[:], Ec[:], tr_nm(Ec[:]), Alu.add)
                Lr = L[:].rearrange("p (n m) -> p n m", n=4)
                mx = tiny("mx", 2, 4)
                nc.vector.tensor_reduce(mx[:], Lr, mybir.AxisListType.X,
                                        Alu.max)
                xm = tiny("xm", 2, 16)
                nc.vector.tensor_tensor(xm[:], L[:], bc_n(mx[:]),
                                        Alu.subtract)
                ex = tiny("ex", 2, 16)
                nc.scalar.activation(ex[:], xm[:], Act.Exp,
                                     bias=actb[0:2, 0:1])
                sm = tiny("sm", 2, 4)
                exr = ex[:].rearrange("p (n m) -> p n m", n=4)
                nc.vector.tensor_reduce(sm[:], exr, mybir.AxisListType.X,
                                        Alu.add)
                rc = tiny("rc", 2, 4)
                nc.vector.reciprocal(rc[:], sm[:])
                S = tiny("S", 2, 16)
                nc.vector.tensor_tensor(S[:], ex[:], bc_n(rc[:]), Alu.mult)

                Sr = S[:].rearrange("p (n m) -> p n m", n=4)
                lo = tiny("lo", 2, 8)
                lor = lo[:].rearrange("p (n m) -> p n m", n=4)
                hi = tiny("hi", 2, 8)
                hir = hi[:].rearrange("p (n m) -> p n m", n=4)
                nc.vector.tensor_tensor(lor, Sr[:, :, 0::2], Sr[:, :, 1::2],
                                        Alu.min)
                nc.vector.tensor_tensor(hir, Sr[:, :, 0::2], Sr[:, :, 1::2],
                                        Alu.max)
                kth = tiny("kth", 2, 4)
                l2 = tiny("l2", 2, 4)
                nc.vector.tensor_tensor(l2[:], lor[:, :, 0], lor[:, :, 1],
                                        Alu.max)
                h2 = tiny("h2", 2, 4)
                nc.vector.tensor_tensor(h2[:], hir[:, :, 0], hir[:, :, 1],
                                        Alu.min)
                nc.vector.tensor_tensor(kth[:], l2[:], h2[:], Alu.min)
                msk = tiny("msk", 2, 16)
                nc.vector.tensor_tensor(msk[:], S[:], bc_n(kth[:]),
                                        Alu.is_ge)
                Sp = tiny("Sp", 2, 16)
                nc.vector.tensor_tensor(Sp[:], S[:], msk[:], Alu.mult)

                A2 = tiny("A2", 2, 16)
                nc.vector.tensor_tensor(A2[:], Sp[:], tr_nm(Sp[:]), Alu.add)
                rs = tiny("rs", 2, 4)
                A2r = A2[:].rearrange("p (n m) -> p n m", n=4)
                nc.vector.tensor_reduce(rs[:], A2r, mybir.AxisListType.X,
                                        Alu.add)
                lnd = tiny("lnd", 2, 4)
                nc.scalar.activation(lnd[:], rs[:], Act.Ln,
                                     bias=actb[0:2, 1:2], scale=0.5)
                q = tiny("q", 2, 4)
                nc.scalar.activation(q[:], lnd[:], Act.Exp, scale=-0.5,
                                     bias=actb[0:2, 0:1])

                t1 = tiny("t1", 2, 16)
                nc.vector.tensor_tensor(t1[:], A2[:], bc_n(q[:]), Alu.mult)
                OPt = tiny("OPt", 2, 16)
                nc.vector.scalar_tensor_tensor(OPt[:], t1[:], 0.5,
                                               bc_m(q[:]),
                                               Alu.mult, Alu.mult)
                col0 = reap(OPt[:], [[0, 4], [4, 4]])
                t2 = tiny("t2", 2, 16)
                nc.vector.tensor_tensor(t2[:], OPt[:], col0, Alu.mult)
                af = tiny("af", 2, 4)
                t2r = t2[:].rearrange("p (n m) -> p n m", n=4)
                nc.vector.tensor_reduce(af[:], t2r, mybir.AxisListType.X,
                                        Alu.add)
                # al3f [2,4] fpr: cols 1-3 = alpha*cn, col 0 = 0
                al3f = tiny("al3f", 2, 4, FPR)
                nc.vector.tensor_tensor(al3f[:], af[:], csb["cnrow4"][:],
                                        Alu.mult)
                psb = ps_tiny.tile([128, 4], FP, tag="gt")
                nc.tensor.matmul(psb[:], csb["gbc"][:], al3f[:],
                                 start=True, stop=True)
                aap = tiny("aap", 128, 4)
                nc.vector.tensor_copy(aap[:], psb[:])
                hand[("al3f", t)] = al3f

                sw = [swpool.tile([128, 9 * 128], FPR, tag=f"sw{n}",
                                  name=f"sw{n}") for n in range(3)]
                for n in range(3):
                    nc.vector.tensor_scalar_mul(
                        sw[n][:],
                        csb["wnod"][:, n * 9 * 128:(n + 1) * 9 * 128],
                        aap[:, n + 1:n + 2])
                hand[("sw", t)] = sw

            def node_block(tp):
                vno, vnn = (vna, vnb) if tp % 2 == 0 else (vnb, vna)
                o0r = o0tiles[tp % 2][:].rearrange("p (h w) -> p h w", h=34)
                al3f = hand.pop(("al3f", tp))
                sw = hand.pop(("sw", tp))
                f04 = hand.pop(("f04", tp))
                sn = wpool.tile([128, 3072], FP, tag="sn")
                snsum = tiny("snsum", 128, 3)
                snsumB = tiny("snsumB", 128, 3)
                for n in range(3):
                    for c in range(2):
                        psn = ps_node.tile([128, 512], FP, tag="pn")
                        for k in range(9):
                            dy, dx = k // 3, k % 3
                            rhs = o0r[:, dy + 16 * c: dy + 16 * c + 16,
                                      dx:dx + 32]
                            nc.tensor.matmul(psn[:],
                                             sw[n][:, k * 128:(k + 1) * 128],
                                             rhs, start=(k == 0),
                                             stop=False)
                        nc.tensor.matmul(
                            psn[:], csb["biasn"][0:1, n * 128:(n + 1) * 128],
                            imA[0:1, 0:512], start=False, stop=False)
                        nc.tensor.matmul(
                            psn[:], csb["apad"][:, n * 128:(n + 1) * 128],
                            reap(al3f[:, n + 1:n + 2], [[0, 512]]),
                            start=False, stop=False, skip_group_check=True)
                        nc.tensor.matmul(
                            psn[:], colmat("in3", n),
                            vno[:, n * 1024 + c * 512:
                                n * 1024 + (c + 1) * 512],
                            start=False, stop=True)
                        sl = slice(n * 1024 + c * 512,
                                   n * 1024 + (c + 1) * 512)
                        nc.scalar.activation(
                            sn[:, sl], psn[:], Act.Sign, bias=mcol[:, 0:1],
                            accum_out=(snsum if c == 0
                                       else snsumB)[:, n:n + 1])
                        nc.vector.scalar_tensor_tensor(
                            vnn[:, sl], sn[:, sl], 0.0, psn[:],
                            Alu.is_lt, Alu.mult)

                # feats + trace update
                psf = ps_tiny.tile([128, 3], FP, tag="gt")
                nc.tensor.matmul(psf[:], colmat("ftmm", 1), snsum[:],
                                 start=True, stop=False)
                nc.tensor.matmul(psf[:], colmat("ftmm", 1), snsumB[:],
                                 start=False, stop=True)
                fn04 = tiny("fn04", 128, 3)
                nc.vector.tensor_scalar(fn04[:], psf[:], ftb2ap, 0.0,
                                        Alu.add, op1=Alu.max)
                nc.vector.scalar_tensor_tensor(
                    Tt[:, 0:1], Tt[:, 0:1], DECAY, f04[:], Alu.mult,
                    Alu.add)
                nc.vector.scalar_tensor_tensor(
                    Tt[:, 1:4], Tt[:, 1:4], DECAY, fn04[:], Alu.mult,
                    Alu.add)

                # output y = yw0*S0 + sum_n ywn*sn' + yc
                ysb = wpool.tile([128, 1024], FP, tag="ysb")
                nc.vector.tensor_scalar(ysb[:], o0r[:, 1:33, 1:33],
                                        yw[0], yc, Alu.mult, op1=Alu.add)
                for n in range(3):
                    nc.vector.scalar_tensor_tensor(
                        ysb[:], sn[:, n * 1024:(n + 1) * 1024], yw[n + 1],
                        ysb[:], Alu.mult, Alu.add)
                nc.gpsimd.dma_start(
                    bass.AP(tensor=y, offset=tp * BC * CO * 1024,
                            ap=[[1024, 128], [1, 1024]]),
                    ysb[:])

            # software-pipelined schedule: the serial graph math of step
            # t overlaps the next step's conv0 on PE.
            im2col(0)
            im2col(1)
            for t in range(nt):
                conv_block(t)
                if t + 2 < nt:
                    im2col(t + 2)
                if t > 0:
                    node_block(t - 1)
                graph_block(t)
            node_block(nt - 1)
    if not nc.is_finalized():
        nc.finalize()
    return nc


_NC_CACHE = {}


def _get_nc(nt=T, yw=(0.125, 0.5, 0.5, 0.5), yc=1.0):
    key = (nt, tuple(float(v) for v in yw), float(yc))
    if key not in _NC_CACHE:
        _NC_CACHE[key] = build_nc(nt, yw, yc)
    return _NC_CACHE[key]


def _split_hi_lo(x):
    """Split fp32 x into hi (11 mantissa bits, fp32r-exact) + lo, each
    padded with two zero columns to 66-wide rows."""
    xb = np.ascontiguousarray(x, np.float32)
    hi = (xb.view(np.uint32) & np.uint32(0xFFFFF000)).view(np.float32)
    lo = (xb - hi).astype(np.float32)
    sh = x.shape[:-1] + (66,)
    hip = np.zeros(sh, np.float32)
    lop = np.zeros(sh, np.float32)
    hip[..., :64] = hi
    lop[..., :64] = lo
    return hip, lop


def kernel(**inputs):
    x = np.asarray(inputs["x"], np.float32)
    consts = _host_consts(
        inputs["conv0_w"], inputs["bn0_g"], inputs["bn0_b"], inputs["bn0_m"],
        inputs["bn0_v"], inputs["lif0_w"], inputs["convs_w"], inputs["bns_g"],
        inputs["bns_b"], inputs["bns_m"], inputs["bns_v"], inputs["lifs_w"],
        inputs["ft_w"], inputs["ft_b"], inputs["gat_w"], inputs["gat_a"],
        inputs["out_weights"])
    consts = {k: np.ascontiguousarray(v, np.float32)
              for k, v in consts.items()}
    sigw = 1.0 / (1.0 + np.exp(-np.asarray(inputs["out_weights"], np.float64)))
    yw = (float(sigw[0]) / 8.0, float(sigw[1]) / 2.0, float(sigw[2]) / 2.0,
          float(sigw[3]) / 2.0)
    yc = float(sigw[0] / 2.0 + (sigw[1] + sigw[2] + sigw[3]) / 2.0)
    nc = _get_nc(T, yw, yc)
    xhi, xlo = _split_hi_lo(x)
    core_ids = list(range(NCORES))
    in_maps = []
    for k in core_ids:
        m = dict(consts)
        m["xh"] = np.ascontiguousarray(xhi[:, k * BC:(k + 1) * BC])
        m["xl"] = np.ascontiguousarray(xlo[:, k * BC:(k + 1) * BC])
        in_maps.append(m)
    res = run_bass_kernel_spmd(nc, in_maps, core_ids).results
    out = np.concatenate([res[k]["y"] for k in core_ids], axis=1)
    return out.astype(np.float32)


# revision 47
# speedup vs baseline: 1.7988x; 1.0948x over previous
"""STSPBlock Trainium2 kernel.

Structure (per core, batch-sharded B=16 -> 8 cores x B=2):
  partitions p = b*64 + channel for all activation tensors.

  - conv0+bn+LIF-input-scale folded into one K=73 im2col matmul.
    x is split HOST-side into hi (11 mantissa bits, exactly
    representable under the PE's float32r moving-operand rounding) and
    lo (residual): 36 hi tap rows + 36 lo tap rows + 1 ones row
    (bias), so the fp32r matmul is numerically exact for the conv.
    Taps are DMA'd from DRAM copies of x_hi/x_lo laid out as
    zero-PADDED 66x66 planes, so no edge-correction matmuls are
    needed. The LIF state add (1-c0)*v rides the same PSUM group via a
    scaled-identity fp32r matmul.

  - Spikes are computed on the SCALAR engine as s' = Sign(u - 1) in
    {-1,+1} ("sign encoding", s = (s'+1)/2); every consumer is linear
    in s, so the affine fix is folded host-side into the next conv's
    weights/bias, the feat-transform consts, and the y weights. The
    out0 pool pad cells are preset to -4 (the sign encoding of 0).
    Sign/Exp/Ln/Copy all live in one ACT table set
    (natural_log_exp_and_others), preloaded once.

  - reset v' = (s' < 0) * u on DVE; avgpool via paired adds (values
    become integer sums in [-4,4] -> exact under fp32r).

  - BETA=0 => S-state is just alpha each step. alpha is applied by
    scaling the block-diag node conv weights; the sign-encoding
    constant term (alpha-dependent, spatially uniform thanks to the -4
    padding) enters PSUM via a tiny K=2 matmul whose rhs is alpha
    broadcast with a 0-stride AP.

  - y = affine combination of sign spikes on DVE.

All bn/LIF/sigmoid parameter folding is done host-side from the actual
input values at call time, so the kernel is fully general.
"""

import numpy as np

import concourse.bass as bass
import concourse.bacc as bacc
import concourse.mybir as mybir
from concourse.tile import TileContext
from concourse.bass_utils import run_bass_kernel_spmd

FP = mybir.dt.float32
FPR = mybir.dt.float32r
Alu = mybir.AluOpType
Act = mybir.ActivationFunctionType

T, BFULL, CIN, H, W = 8, 16, 2, 64, 64
CO, NN, HEADS = 64, 4, 4
HP, WP = 32, 32
BC = 2                    # batch per core
NCORES = 8
EPS = 1e-5
DECAY = 0.6
HD = CO // HEADS          # 16
PL = 66 * 66              # padded plane size
NPL = 4                   # planes per timestep (b, ci)

ACT_SET_NLE = None  # index of the Sign+Exp+Ln+Copy ACT table set


# ----------------------------------------------------------------- host consts
def _host_consts(conv0_w, bn0_g, bn0_b, bn0_m, bn0_v, lif0_w,
                 convs_w, bns_g, bns_b, bns_m, bns_v, lifs_w,
                 ft_w, ft_b, gat_w, gat_a, out_weights):
    f32 = np.float32
    sig = lambda z: 1.0 / (1.0 + np.exp(-z.astype(np.float64)))
    c0 = f32(sig(lif0_w))
    cn = sig(lifs_w).astype(f32)          # [3]

    s0c = (bn0_g / np.sqrt(bn0_v + EPS)).astype(f32)
    bias0 = ((bn0_b - bn0_m * s0c) * c0).astype(f32)
    W0f = (conv0_w * s0c[:, None, None, None] * c0).astype(f32)  # [64,2,3,3]

    # fp32r rounds BOTH matmul operands to ~12 mantissa bits. Split
    # weights and x into hi (11-bit, exactly representable) + lo parts
    # and keep the three cross products Whi*xhi + Whi*xlo + Wlo*xhi
    # (dropped Wlo*xlo term is ~2^-22 relative): the conv is exact.
    def tr11(a):
        return (np.ascontiguousarray(a, f32).view(np.uint32)
                & np.uint32(0xFFFFF000)).view(f32)

    W0hi = tr11(W0f)
    W0lo = (W0f - W0hi).astype(f32)
    b0hi = tr11(bias0)
    b0lo = (bias0 - b0hi).astype(f32)

    # w0bd [110, 3*128]: rows 0/1 = bias hi/lo (ones taps); rows 2-37 =
    # Whi (vs x_hi rows), 38-73 = Whi (vs x_lo), 74-109 = Wlo (vs x_hi
    # again). col m = v*128 + b*64 + co. x rows are host-padded to 66
    # wide with zero pad columns, so horizontal tap overflow reads
    # zeros; only vertical overflow needs fixing: variant 1 subtracts
    # the dy=0 taps' garbage at h=0, variant 2 the dy=2 taps' at h=63.
    w0bd = np.zeros((110, 3 * 128), f32)
    w0bd[0, 0:64] = b0hi
    w0bd[0, 64:128] = b0hi
    w0bd[1, 0:64] = b0lo
    w0bd[1, 64:128] = b0lo

    def put(v, dy, dx, sgn):
        for b in range(2):
            for ci in range(2):
                p = dy * 12 + dx * 4 + b * 2 + ci
                c0_, c1_ = v * 128 + b * 64, v * 128 + (b + 1) * 64
                w0bd[2 + p, c0_:c1_] = sgn * W0hi[:, ci, dy, dx]
                w0bd[38 + p, c0_:c1_] = sgn * W0hi[:, ci, dy, dx]
                w0bd[74 + p, c0_:c1_] = sgn * W0lo[:, ci, dy, dx]

    for dy in range(3):
        for dx in range(3):
            put(0, dy, dx, 1.0)
    for dx in range(3):
        put(1, 0, dx, -1.0)
        put(2, 2, dx, -1.0)

    i0 = ((1.0 - c0) * np.eye(128)).astype(f32)

    sncol = (bns_g / np.sqrt(bns_v + EPS)).astype(f32)            # [3,64]
    biasn_raw = (bns_b - bns_m * sncol).astype(f32)               # [3,64]
    # 0.25 = avgpool fold; extra 0.5 = sign-encoding decode s=(s'+1)/2
    Wf = (convs_w * sncol[:, :, None, None, None] * 0.25).astype(f32)
    Wh = (Wf * 0.5).astype(f32)

    # wnod [3, 9, 128, 128]: per (node, tap) block-diag lhsT over b
    wnod = np.zeros((3, 9, 128, 128), f32)
    for n in range(3):
        for dy in range(3):
            for dx in range(3):
                k = dy * 3 + dx
                blk = Wh[n, :, :, dy, dx].T    # [ci, co]
                wnod[n, k, 0:64, 0:64] = blk
                wnod[n, k, 64:128, 64:128] = blk

    in3 = np.stack([(1.0 - cn[n]) * np.eye(128) for n in range(3)]).astype(f32)

    # bap [3, 3*128]: per-node combined bias lhsT. Row 0 pairs with a
    # ones rhs row (static bn bias); rows 1/2 pair with alpha*cn rhs
    # rows and carry the sign-encoding uniform term 2*sum_taps(Wf).
    tapsum = Wf.sum(axis=(2, 3, 4))            # [3, 64]
    bap = np.zeros((3, 3 * 128), f32)
    for n in range(3):
        bap[2, n * 128:(n + 1) * 128] = np.tile(cn[n] * biasn_raw[n], 2)
        for b in range(2):
            bap[b, n * 128 + b * 64:n * 128 + (b + 1) * 64] = \
                2.0 * tapsum[n]

    def bd(m):  # block-diag [128,128] of m.T twice ([co,ci] -> lhsT)
        z = np.zeros((128, 128), f32)
        z[0:64, 0:64] = m.T
        z[64:128, 64:128] = m.T
        return z

    # feat transform: f04 = 0.4*relu(ftw @ mean + ftb), sign-decode and
    # the 0.4 trace factor folded:  mean0 = (0.125/1024)*S0sum + 0.5,
    # meann = (0.5/1024)*Snsum + 0.5.
    ftmm = np.stack([bd(ft_w * (0.4 * 0.125 / 1024.0)),
                     bd(ft_w * (0.4 * 0.5 / 1024.0))])
    ftb_f = (0.4 * (ft_b + 0.5 * ft_w.sum(axis=1))).astype(f32)
    ftb2 = np.tile(ftb_f, 2).reshape(128, 1).astype(f32)
    gwbd = bd(gat_w).astype(f32)

    # ga1/ga2 [128, 8]: in p=(b, c') c'=h*16+d ; out m = b*4+h
    ga1 = np.zeros((128, 8), f32)
    ga2 = np.zeros((128, 8), f32)
    for b in range(2):
        for h in range(HEADS):
            for d in range(HD):
                ga1[b * 64 + h * 16 + d, b * 4 + h] = gat_a[h, d]
                ga2[b * 64 + h * 16 + d, b * 4 + h] = gat_a[h, HD + d]

    # ghbd [8,2]: p=(b,h) -> col b ; carries 0.5(sym)*0.25(mean h)/0.01(temp)
    ghbd = np.zeros((8, 2), f32)
    for b in range(2):
        ghbd[b * 4:(b + 1) * 4, b] = 12.5

    gbc = np.zeros((2, 128), f32)
    gbc[0, 0:64] = 1.0
    gbc[1, 64:128] = 1.0

    # cnrow4 [2,4]: col 0 unused (node 0 has no conv), cols 1-3 = cn
    cnrow4 = np.zeros((2, 4), f32)
    cnrow4[:, 1:4] = cn[None, :]

    def cols(stk):  # [k,128,128] -> [128, k*128]
        return np.ascontiguousarray(
            np.transpose(stk, (1, 0, 2)).reshape(128, -1))

    return dict(w0bd=w0bd, i0=i0, wnod=cols(wnod.reshape(27, 128, 128)),
                in3=cols(in3), bap=bap,
                ftmm=cols(ftmm), ftb2=ftb2, gwbd=gwbd,
                ga1=ga1, ga2=ga2, ghbd=ghbd, gbc=gbc, cnrow4=cnrow4)


CONST_SHAPES = dict(w0bd=(110, 3 * 128), i0=(128, 128), wnod=(128, 27 * 128),
                    in3=(128, 3 * 128), bap=(3, 3 * 128),
                    ftmm=(128, 2 * 128), ftb2=(128, 1), gwbd=(128, 128),
                    ga1=(128, 8), ga2=(128, 8), ghbd=(8, 2), gbc=(2, 128),
                    cnrow4=(2, 4))
# consts that feed the big float32r matmuls
FPR_CONSTS = {"w0bd", "i0", "wnod", "in3", "bap", "gbc"}


def _act_set_id():
    global ACT_SET_NLE
    if ACT_SET_NLE is None:
        from concourse.hw_specs import get_activation_tables
        for i, name in enumerate(get_activation_tables("gen3")):
            if name == "natural_log_exp_and_others":
                ACT_SET_NLE = i
                break
        assert ACT_SET_NLE is not None
    return ACT_SET_NLE


# ------------------------------------------------------------------ the module
def build_nc(nt=T, yw=(0.125, 0.5, 0.5, 0.5), yc=1.0):
    nc = bacc.Bacc(None, target_bir_lowering=False)
    xh = nc.declare_dram_parameter("xh", [T, BC, CIN, H, 66], FPR,
                                   isOutput=False)
    xl = nc.declare_dram_parameter("xl", [T, BC, CIN, H, 66], FPR,
                                   isOutput=False)
    cst = {k: nc.declare_dram_parameter(
               k, list(v), FPR if k in FPR_CONSTS else FP, isOutput=False)
           for k, v in CONST_SHAPES.items()}
    y = nc.declare_dram_parameter("y", [T, BC, CO, HP, WP], FP, isOutput=True)
    xlinH = nc.dram_tensor("xlinH", [T * 16896 + 256], FPR)
    xlinL = nc.dram_tensor("xlinL", [T * 16896 + 256], FPR)

    with TileContext(nc) as tc:
        with (
            tc.tile_pool(name="consts", bufs=1) as cpool,
            tc.tile_pool(name="state", bufs=1) as spool,
            tc.tile_pool(name="im", bufs=1) as impool,
            tc.tile_pool(name="work", bufs=2) as wpool,
            tc.tile_pool(name="sw", bufs=1) as swpool,
            tc.tile_pool(name="tiny", bufs=3) as tpool,
            tc.tile_pool(name="pconv", bufs=3, space="PSUM") as ps_conv,
            tc.tile_pool(name="pnode", bufs=2, space="PSUM") as ps_node,
            tc.tile_pool(name="ptiny", bufs=3, space="PSUM") as ps_tiny,
        ):
            # ---- preload the one ACT table set we use (Sign/Exp/Ln/Copy)
            ld = mybir.InstLoadActFuncSet(
                name=f"I-{nc.next_id()}", ins=[], outs=[],
                act_func_set_id=_act_set_id())
            nc.scalar.add_instruction(ld)

            # ---- consts to SBUF
            csb = {}
            for k, shp in CONST_SHAPES.items():
                t_ = cpool.tile(list(shp), FPR if k in FPR_CONSTS else FP,
                                tag=k)
                nc.sync.dma_start(t_[:], cst[k][:])
                csb[k] = t_

            zcol = cpool.tile([128, 1], FP, tag="zcol")
            nc.vector.memset(zcol[:], 0.0)
            ocol = cpool.tile([128, 1], FP, tag="ocol")
            nc.vector.memset(ocol[:], 1.0)
            m4col = cpool.tile([128, 1], FP, tag="m4col")
            nc.vector.memset(m4col[:], -4.0)
            mcol = cpool.tile([128, 1], FP, tag="mcol")   # ACT bias -1
            nc.vector.memset(mcol[:], -1.0)
            actb = cpool.tile([128, 2], FP, tag="actb")
            nc.vector.memset(actb[:, 0:1], 0.0)
            nc.vector.memset(actb[:, 1:2], 1e-6)

            def bcfill(dst, src2d, *shape):
                nc.vector.tensor_copy(
                    dst, bass.AP(tensor=src2d.tensor, offset=src2d.offset,
                                 ap=[list(src2d.ap[0])]
                                 + [[0, s] for s in shape]))

            # ---- states
            v0a = spool.tile([128, 4096], FPR, tag="v0a")
            v0b = spool.tile([128, 4096], FPR, tag="v0b")
            vna = spool.tile([128, 3072], FPR, tag="vna")
            vnb = spool.tile([128, 3072], FPR, tag="vnb")
            Tt = spool.tile([128, 4], FP, tag="Tt")
            bcfill(v0a[:], zcol[:, 0:1], 4096)
            bcfill(vna[:], zcol[:, 0:1], 3072)
            nc.vector.memset(Tt[:], 0.0)

            # ---- persistent padded out0 tiles, pads preset to -4 once
            o0tiles = []
            for nm in ("o0A", "o0B"):
                o0t = spool.tile([128, 34 * 34], FPR, tag=nm)
                o0v = o0t[:].rearrange("p (h w) -> p h w", h=34)
                bcfill(o0v[:, 0, :], m4col[:, 0:1], 34)
                bcfill(o0v[:, 33, :], m4col[:, 0:1], 34)
                bcfill(o0v[:, 1:33, 0:1], m4col[:, 0:1], 32, 1)
                bcfill(o0v[:, 1:33, 33:34], m4col[:, 0:1], 32, 1)
                o0tiles.append(o0t)

            # ---- bias rhs [3,4]: rows 0-1 get alpha*cn per step
            # (plain copy from al3f); row 2 = ones (static).
            bias_rhs = spool.tile([3, 4], FPR, tag="bias_rhs")
            bcfill(bias_rhs[:], ocol[0:3, 0:1], 4)

            # ---- x_hi/x_lo -> flat xlin (128-elem zero pad head/tail)
            zrow = cpool.tile([1, 128], FPR, tag="zrow")
            bcfill(zrow[:], zcol[0:1, 0:1], 128)
            for xsrc, xdst in ((xh, xlinH), (xl, xlinL)):
                nc.sync.dma_start(
                    bass.AP(tensor=xdst, offset=0,
                            ap=[[128, 1], [1, 128]]),
                    zrow[:])
                nc.sync.dma_start(
                    bass.AP(tensor=xdst, offset=128 + T * 16896,
                            ap=[[128, 1], [1, 128]]),
                    zrow[:])
                nc.sync.dma_start(
                    bass.AP(tensor=xdst, offset=128,
                            ap=[[16896, T], [1, 16896]]),
                    bass.AP(tensor=xsrc, offset=0,
                            ap=[[16896, T], [1, 16896]]))

            # ---- im2col tiles (row 0 = ones, set once; the rest streamed)
            imA = impool.tile([110, 4224], FPR, tag="imA")
            imB = impool.tile([110, 4224], FPR, tag="imB")
            for imt in (imA, imB):
                bcfill(imt[0:2, :], ocol[0:2, 0:1], 4224)

            def colmat(name, j):
                return csb[name][:, j * 128:(j + 1) * 128]
            ftb2ap = csb["ftb2"][:]

            def tiny(tag, p_, f_, dt_=FP):
                return tpool.tile([p_, f_], dt_, tag=tag, name=tag)

            def reap(ap_, tail):
                dims = [list(d) for d in ap_.ap][:-1] + tail
                return bass.AP(tensor=ap_.tensor, offset=ap_.offset,
                               ap=dims)

            def bc_n(ap_):  # [p,4] -> free (n,m): n varies, m bcast
                return reap(ap_, [[1, 4], [0, 4]])

            def bc_m(ap_):  # free (n,m): n bcast, m varies
                return reap(ap_, [[0, 4], [1, 4]])

            def tr_nm(ap_):  # read transposed over (n,m)
                return reap(ap_, [[1, 4], [4, 4]])

            def im2col(t):
                im = imA if t % 2 == 0 else imB
                for dy in range(3):
                    for base, xlin_ in ((2, xlinH), (38, xlinL),
                                        (74, xlinH)):
                        p0 = base + dy * 12
                        nc.sync.dma_start(
                            im[p0:p0 + 12, :],
                            bass.AP(tensor=xlin_,
                                    offset=(128 + t * 16896
                                            + (dy - 1) * 66 - 1),
                                    ap=[[1, 3], [4224, 4], [1, 4224]]))

            # per-step tiles handed from graph(t) to node(t)
            hand = {}

            def conv_block(t):
                v0o, v0n = (v0a, v0b) if t % 2 == 0 else (v0b, v0a)
                im = imA if t % 2 == 0 else imB
                imv = im[:].rearrange("p (h w) -> p h w", h=64)
                o0r = o0tiles[t % 2][:].rearrange("p (h w) -> p h w", h=34)
                p1 = wpool.tile([128, 2048], FP, tag="p1")
                Wv = lambda v: csb["w0bd"][:, v * 128:(v + 1) * 128]
                for c in range(8):
                    sl = slice(c * 512, (c + 1) * 512)
                    ps = ps_conv.tile([128, 512], FP, tag="pc")
                    nc.tensor.matmul(ps[:], Wv(0),
                                     imv[:, c * 8:(c + 1) * 8, 0:64],
                                     start=True, stop=False)
                    if c == 0:
                        # subtract dy=0 taps' vertical-overflow garbage
                        nc.tensor.matmul(ps[:, 0:64], Wv(1),
                                         imv[:, 0:1, 0:64],
                                         start=False, stop=False,
                                         skip_group_check=True)
                    if c == 7:
                        nc.tensor.matmul(ps[:, 448:512], Wv(2),
                                         imv[:, 63:64, 0:64],
                                         start=False, stop=False,
                                         skip_group_check=True)
                    nc.tensor.matmul(ps[:], csb["i0"][:], v0o[:, sl],
                                     start=False, stop=True,
                                     skip_group_check=True)
                    # s' = Sign(u-1) on ACT ; v' = (s'<0)*u on DVE
                    s0c = wpool.tile([128, 512], FP, tag="s0c")
                    nc.scalar.activation(s0c[:], ps[:], Act.Sign,
                                         bias=mcol[:, 0:1], scale=1.0)
                    nc.vector.scalar_tensor_tensor(
                        v0n[:, sl], s0c[:], 0.0, ps[:], Alu.is_lt, Alu.mult)
                    s0r = s0c[:].rearrange("p (h w) -> p h w", h=8)
                    p1r = p1[:].rearrange("p (h w) -> p h w", h=64)
                    nc.vector.tensor_tensor(
                        p1r[:, c * 8:(c + 1) * 8, :],
                        s0r[:, :, 0::2], s0r[:, :, 1::2], Alu.add)

                # pool rows into padded out0 (S in [-4,4]) + f0 sum
                f0sum = tiny("f0sum", 128, 1)
                p1v = p1[:].rearrange("p (h w) -> p h w", h=64)
                nc.vector.tensor_tensor(
                    o0r[:, 1:33, 1:33], p1v[:, 0::2, :], p1v[:, 1::2, :],
                    Alu.add)
                nc.vector.tensor_reduce(f0sum[:], o0r[:, 1:33, 1:33],
                                        mybir.AxisListType.XY, Alu.add)
                hand[("f0sum", t)] = f0sum

            def graph_block(t):
                f0sum = hand.pop(("f0sum", t))
                psf0 = ps_tiny.tile([128, 1], FP, tag="gt")
                nc.tensor.matmul(psf0[:], colmat("ftmm", 0), f0sum[:],
                                 start=True, stop=True)
                f04 = tiny("f04", 128, 1)
                nc.vector.tensor_scalar(f04[:], psf0[:], ftb2ap, 0.0,
                                        Alu.add, op1=Alu.max)
                hand[("f04", t)] = f04
                # trace row0 pre-update
                nc.vector.scalar_tensor_tensor(
                    Tt[:, 0:1], Tt[:, 0:1], DECAY, f04[:], Alu.mult, Alu.add)

                psg = ps_tiny.tile([128, 4], FP, tag="gt")
                nc.tensor.matmul(psg[:], csb["gwbd"][:], Tt[:],
                                 start=True, stop=True)
                hpc = tiny("hpc", 128, 4)
                nc.vector.tensor_copy(hpc[:], psg[:])

                pse1 = ps_tiny.tile([8, 4], FP, tag="gt")
                nc.tensor.matmul(pse1[:], csb["ga1"][:], hpc[:],
                                 start=True, stop=True)
                e1t = tiny("e1t", 8, 4)
                nc.vector.tensor_copy(e1t[:], pse1[:])
                pse2 = ps_tiny.tile([8, 4], FP, tag="gt")
                nc.tensor.matmul(pse2[:], csb["ga2"][:], hpc[:],
                                 start=True, stop=True)
                e2t = tiny("e2t", 8, 4)
                nc.vector.tensor_copy(e2t[:], pse2[:])

                es = tiny("es", 8, 16)
                nc.vector.tensor_tensor(es[:], bc_n(e1t[:]), bc_m(e2t[:]),
                                        Alu.add)
                el = tiny("el", 8, 16)
                nc.vector.scalar_tensor_tensor(el[:], es[:], 0.2, es[:],
                                               Alu.mult, Alu.max)

                psE = ps_tiny.tile([2, 16], FP, tag="gt")
                nc.tensor.matmul(psE[:], csb["ghbd"][:], el[:],
                                 start=True, stop=True)
                Ec = tiny("Ec", 2, 16)
                nc.vector.tensor_copy(Ec[:], psE[:])

                L = tiny("L", 2, 16)
                nc.vector.tensor_tensor(L[:], Ec[:], tr_nm(Ec[:]), Alu.add)
                Lr = L[:].rearrange("p (n m) -> p n m", n=4)
                mx = tiny("mx", 2, 4)
                nc.vector.tensor_reduce(mx[:], Lr, mybir.AxisListType.X,
                                        Alu.max)
                xm = tiny("xm", 2, 16)
                nc.vector.tensor_tensor(xm[:], L[:], bc_n(mx[:]),
                                        Alu.subtract)
                ex = tiny("ex", 2, 16)
                nc.scalar.activation(ex[:], xm[:], Act.Exp,
                                     bias=actb[0:2, 0:1])
                sm = tiny("sm", 2, 4)
                exr = ex[:].rearrange("p (n m) -> p n m", n=4)
                nc.vector.tensor_reduce(sm[:], exr, mybir.AxisListType.X,
                                        Alu.add)
                rc = tiny("rc", 2, 4)
                nc.vector.reciprocal(rc[:], sm[:])
                S = tiny("S", 2, 16)
                nc.vector.tensor_tensor(S[:], ex[:], bc_n(rc[:]), Alu.mult)

                Sr = S[:].rearrange("p (n m) -> p n m", n=4)
                lo = tiny("lo", 2, 8)
                lor = lo[:].rearrange("p (n m) -> p n m", n=4)
                hi = tiny("hi", 2, 8)
                hir = hi[:].rearrange("p (n m) -> p n m", n=4)
                nc.vector.tensor_tensor(lor, Sr[:, :, 0::2], Sr[:, :, 1::2],
                                        Alu.min)
                nc.vector.tensor_tensor(hir, Sr[:, :, 0::2], Sr[:, :, 1::2],
                                        Alu.max)
                kth = tiny("kth", 2, 4)
                l2 = tiny("l2", 2, 4)
                nc.vector.tensor_tensor(l2[:], lor[:, :, 0], lor[:, :, 1],
                                        Alu.max)
                h2 = tiny("h2", 2, 4)
                nc.vector.tensor_tensor(h2[:], hir[:, :, 0], hir[:, :, 1],
                                        Alu.min)
                nc.vector.tensor_tensor(kth[:], l2[:], h2[:], Alu.min)
                msk = tiny("msk", 2, 16)
                nc.vector.tensor_tensor(msk[:], S[:], bc_n(kth[:]),
                                        Alu.is_ge)
                Sp = tiny("Sp", 2, 16)
                nc.vector.tensor_tensor(Sp[:], S[:], msk[:], Alu.mult)

                A2 = tiny("A2", 2, 16)
                nc.vector.tensor_tensor(A2[:], Sp[:], tr_nm(Sp[:]), Alu.add)
                rs = tiny("rs", 2, 4)
                A2r = A2[:].rearrange("p (n m) -> p n m", n=4)
                nc.vector.tensor_reduce(rs[:], A2r, mybir.AxisListType.X,
                                        Alu.add)
                lnd = tiny("lnd", 2, 4)
                nc.scalar.activation(lnd[:], rs[:], Act.Ln,
                                     bias=actb[0:2, 1:2], scale=0.5)
                q = tiny("q", 2, 4)
                nc.scalar.activation(q[:], lnd[:], Act.Exp, scale=-0.5,
                                     bias=actb[0:2, 0:1])

                t1 = tiny("t1", 2, 16)
                nc.vector.tensor_tensor(t1[:], A2[:], bc_n(q[:]), Alu.mult)
                OPt = tiny("OPt", 2, 16)
                nc.vector.scalar_tensor_tensor(OPt[:], t1[:], 0.5,
                                               bc_m(q[:]),
                                               Alu.mult, Alu.mult)
                col0 = reap(OPt[:], [[0, 4], [4, 4]])
                t2 = tiny("t2", 2, 16)
                nc.vector.tensor_tensor(t2[:], OPt[:], col0, Alu.mult)
                af = tiny("af", 2, 4)
                t2r = t2[:].rearrange("p (n m) -> p n m", n=4)
                nc.vector.tensor_reduce(af[:], t2r, mybir.AxisListType.X,
                                        Alu.add)
                # al3f [2,4] fpr: cols 1-3 = alpha*cn, col 0 = 0
                al3f = tiny("al3f", 2, 4, FPR)
                nc.vector.tensor_tensor(al3f[:], af[:], csb["cnrow4"][:],
                                        Alu.mult)
                psb = ps_tiny.tile([128, 4], FP, tag="gt")
                nc.tensor.matmul(psb[:], csb["gbc"][:], al3f[:],
                                 start=True, stop=True)
                aap = tiny("aap", 128, 4)
                nc.vector.tensor_copy(aap[:], psb[:])
                nc.vector.tensor_copy(bias_rhs[0:2, :], al3f[:])

                sw = [swpool.tile([128, 9 * 128], FPR, tag=f"sw{n}",
                                  name=f"sw{n}") for n in range(3)]
                for n in range(3):
                    nc.vector.tensor_scalar_mul(
                        sw[n][:],
                        csb["wnod"][:, n * 9 * 128:(n + 1) * 9 * 128],
                        aap[:, n + 1:n + 2])
                hand[("sw", t)] = sw

            def node_block(tp):
                vno, vnn = (vna, vnb) if tp % 2 == 0 else (vnb, vna)
                o0r = o0tiles[tp % 2][:].rearrange("p (h w) -> p h w", h=34)
                sw = hand.pop(("sw", tp))
                f04 = hand.pop(("f04", tp))
                sn = wpool.tile([128, 3072], FP, tag="sn")
                snsum = tiny("snsum", 128, 3)
                snsumB = tiny("snsumB", 128, 3)
                for n in range(3):
                    for c in range(2):
                        psn = ps_node.tile([128, 512], FP, tag="pn")
                        for k in range(9):
                            dy, dx = k // 3, k % 3
                            rhs = o0r[:, dy + 16 * c: dy + 16 * c + 16,
                                      dx:dx + 32]
                            nc.tensor.matmul(psn[:],
                                             sw[n][:, k * 128:(k + 1) * 128],
                                             rhs, start=(k == 0),
                                             stop=False)
                        nc.tensor.matmul(
                            psn[:], csb["bap"][:, n * 128:(n + 1) * 128],
                            reap(bias_rhs[:, n + 1:n + 2], [[0, 512]]),
                            start=False, stop=False, skip_group_check=True)
                        nc.tensor.matmul(
                            psn[:], colmat("in3", n),
                            vno[:, n * 1024 + c * 512:
                                n * 1024 + (c + 1) * 512],
                            start=False, stop=True)
                        sl = slice(n * 1024 + c * 512,
                                   n * 1024 + (c + 1) * 512)
                        nc.scalar.activation(
                            sn[:, sl], psn[:], Act.Sign, bias=mcol[:, 0:1],
                            accum_out=(snsum if c == 0
                                       else snsumB)[:, n:n + 1])
                        nc.vector.scalar_tensor_tensor(
                            vnn[:, sl], sn[:, sl], 0.0, psn[:],
                            Alu.is_lt, Alu.mult)

                # feats + trace update
                psf = ps_tiny.tile([128, 3], FP, tag="gt")
                nc.tensor.matmul(psf[:], colmat("ftmm", 1), snsum[:],
                                 start=True, stop=False)
                nc.tensor.matmul(psf[:], colmat("ftmm", 1), snsumB[:],
                                 start=False, stop=True)
                fn04 = tiny("fn04", 128, 3)
                nc.vector.tensor_scalar(fn04[:], psf[:], ftb2ap, 0.0,
                                        Alu.add, op1=Alu.max)
                nc.vector.scalar_tensor_tensor(
                    Tt[:, 0:1], Tt[:, 0:1], DECAY, f04[:], Alu.mult,
                    Alu.add)
                nc.vector.scalar_tensor_tensor(
                    Tt[:, 1:4], Tt[:, 1:4], DECAY, fn04[:], Alu.mult,
                    Alu.add)

                # output y = yw0*S0 + sum_n ywn*sn' + yc
                ysb = wpool.tile([128, 1024], FP, tag="ysb")
                nc.vector.tensor_scalar(ysb[:], o0r[:, 1:33, 1:33],
                                        yw[0], yc, Alu.mult, op1=Alu.add)
                for n in range(3):
                    nc.vector.scalar_tensor_tensor(
                        ysb[:], sn[:, n * 1024:(n + 1) * 1024], yw[n + 1],
                        ysb[:], Alu.mult, Alu.add)
                nc.scalar.dma_start(
                    bass.AP(tensor=y, offset=tp * BC * CO * 1024,
                            ap=[[1024, 128], [1, 1024]]),
                    ysb[:])

            # software-pipelined schedule: the serial graph math of step
            # t overlaps the next step's conv0 on PE.
            im2col(0)
            im2col(1)
            for t in range(nt):
                conv_block(t)
                if t + 2 < nt:
                    im2col(t + 2)
                if t > 0:
                    node_block(t - 1)
                graph_block(t)
            node_block(nt - 1)
    if not nc.is_finalized():
        nc.finalize()
    return nc


_NC_CACHE = {}


def _get_nc(nt=T, yw=(0.125, 0.5, 0.5, 0.5), yc=1.0):
    key = (nt, tuple(float(v) for v in yw), float(yc))
    if key not in _NC_CACHE:
        _NC_CACHE[key] = build_nc(nt, yw, yc)
    return _NC_CACHE[key]


def _split_hi_lo(x):
    """Split fp32 x into hi (11 mantissa bits, fp32r-exact) + lo, each
    padded with two zero columns to 66-wide rows."""
    xb = np.ascontiguousarray(x, np.float32)
    hi = (xb.view(np.uint32) & np.uint32(0xFFFFF000)).view(np.float32)
    lo = (xb - hi).astype(np.float32)
    sh = x.shape[:-1] + (66,)
    hip = np.zeros(sh, np.float32)
    lop = np.zeros(sh, np.float32)
    hip[..., :64] = hi
    lop[..., :64] = lo
    return hip, lop


def kernel(**inputs):
    x = np.asarray(inputs["x"], np.float32)
    consts = _host_consts(
        inputs["conv0_w"], inputs["bn0_g"], inputs["bn0_b"], inputs["bn0_m"],
        inputs["bn0_v"], inputs["lif0_w"], inputs["convs_w"], inputs["bns_g"],
        inputs["bns_b"], inputs["bns_m"], inputs["bns_v"], inputs["lifs_w"],
        inputs["ft_w"], inputs["ft_b"], inputs["gat_w"], inputs["gat_a"],
        inputs["out_weights"])
    consts = {k: np.ascontiguousarray(v, np.float32)
              for k, v in consts.items()}
    sigw = 1.0 / (1.0 + np.exp(-np.asarray(inputs["out_weights"], np.float64)))
    yw = (float(sigw[0]) / 8.0, float(sigw[1]) / 2.0, float(sigw[2]) / 2.0,
          float(sigw[3]) / 2.0)
    yc = float(sigw[0] / 2.0 + (sigw[1] + sigw[2] + sigw[3]) / 2.0)
    nc = _get_nc(T, yw, yc)
    xhi, xlo = _split_hi_lo(x)
    core_ids = list(range(NCORES))
    in_maps = []
    for k in core_ids:
        m = dict(consts)
        m["xh"] = np.ascontiguousarray(xhi[:, k * BC:(k + 1) * BC])
        m["xl"] = np.ascontiguousarray(xlo[:, k * BC:(k + 1) * BC])
        in_maps.append(m)
    res = run_bass_kernel_spmd(nc, in_maps, core_ids).results
    out = np.concatenate([res[k]["y"] for k in core_ids], axis=1)
    return out.astype(np.float32)
